# revision 2
# baseline (speedup 1.0000x reference)
import os
import numpy as np
import ml_dtypes

import concourse.bass as bass
import concourse.tile as tile
from concourse import bacc, mybir
from concourse.bass_utils import run_bass_kernel_spmd

L, B, Q, D, NC, CS = 6, 32, 900, 256, 10, 10
EPS = 1e-5
NCORES = 8
BPC = B // NCORES          # 4 samples per core
T = BPC * Q                # 3600 tokens per core
NT = 8                     # token tiles per layer
N = 464                    # tokens per tile
TP = NT * N                # 3712 padded tokens
BF16 = mybir.dt.bfloat16
F32 = mybir.dt.float32
FP8 = mybir.dt.float8e4
AF = mybir.ActivationFunctionType
ALU = mybir.AluOpType
DR = mybir.MatmulPerfMode.DoubleRow
NPF8 = ml_dtypes.float8_e4m3

# head-channel permutation: rows 0,1,2 = sigmoid channels (orig 0,1,4)
PERM = [0, 1, 4, 2, 3, 5, 6, 7, 8, 9]
INV = list(np.argsort(PERM))       # coord channel c <- row INV[c]

_cache = {}


def _build_fast():
    """fp8 DoubleRow pipeline; requires all linear/LN biases zero.

    Tricks (all exact up to float rounding):
    - LN mean removal folded into centered weights (W - rowmean W).
    - ln_g folded into weight columns; LN1's rstd skipped entirely and
      LN2's rstd pulled through ReLU + head GEMM (LayerNorm is invariant
      to per-token input scaling), applied to the [10,N] head output.
    - E[z^2] computed with per-feature 1/(256 g^2) weights so the folded
      g does not disturb the variance.
    - cls_b3 added as a rank-1 (b3 x sigma) accumulate into the head psum
      before the rstd scale (sigma = 1/rstd from the same bf16 value).
    - fp8 x16 weight scaling cancels through the same invariances; the
      reg branch rescales via x16/x256 biases and w3r/256.
    """
    nc = bacc.Bacc("TRN2", target_bir_lowering=False, debug=False,
                   enable_asserts=False, num_devices=NCORES)
    hsT = nc.dram_tensor("hsT", [L, 2, 128, TP], FP8, kind="ExternalInput").ap()
    wst = nc.dram_tensor("wst", [L, 128, 4, 2, 2, 128], FP8, kind="ExternalInput").ap()
    w3t = nc.dram_tensor("w3t", [128, L, 2, 2, 10], BF16, kind="ExternalInput").ap()
    scal = nc.dram_tensor("scal", [128, L, 2], BF16, kind="ExternalInput").ap()
    b3t = nc.dram_tensor("b3t", [1, L, 10], BF16, kind="ExternalInput").ap()
    hb = nc.dram_tensor("hb", [10, 2 * L + 2], F32, kind="ExternalInput").ap()
    irt = nc.dram_tensor("irt", [32, TP], BF16, kind="ExternalInput").ap()
    selt = nc.dram_tensor("selt", [32, 10 * L], BF16, kind="ExternalInput").ap()
    o_cls = nc.dram_tensor("o_cls", [L, 10, TP], F32, kind="ExternalOutput").ap()
    o_crd = nc.dram_tensor("o_crd", [L, 10, TP], F32, kind="ExternalOutput").ap()

    with tile.TileContext(nc) as tc:
        with (
            tc.tile_pool(name="const", bufs=1) as cp,
            tc.tile_pool(name="stream", bufs=2) as sp,
            tc.tile_pool(name="wk", bufs=3) as wk,
            tc.tile_pool(name="ps", bufs=2, space="PSUM") as pp,
        ):
            onesc = cp.tile([1, 128], BF16)
            nc.vector.memset(onesc[:], 1.0)
            eps1 = cp.tile([1, 1], F32)
            nc.vector.memset(eps1[:], EPS)
            zer = cp.tile([128, 1], F32)
            nc.vector.memset(zer[:], 0.0)

            w_sb = []
            for l in range(L):
                wt = cp.tile([128, 4, 2, 2, 128], FP8, tag=f"w{l}", name=f"w{l}")
                nc.sync.dma_start(wt[:], wst[l])
                w_sb.append(wt)
            w3_sb = cp.tile([128, L, 2, 2, 10], BF16)
            nc.sync.dma_start(w3_sb[:], w3t)
            sc_sb = cp.tile([128, L, 2], BF16)
            nc.sync.dma_start(sc_sb[:], scal)
            b3_sb = cp.tile([1, L, 10], BF16)
            nc.sync.dma_start(b3_sb[:], b3t)
            hb_sb = cp.tile([10, 2 * L + 2], F32)
            nc.sync.dma_start(hb_sb[:], hb)
            ir_sb = cp.tile([32, TP], BF16)
            nc.sync.dma_start(ir_sb[:], irt)
            sel_sb = cp.tile([32, 10 * L], BF16)
            nc.sync.dma_start(sel_sb[:], selt)

            for l in range(L):
                hsl = sp.tile([128, 2, TP], FP8, tag="hs", name=f"hs{l}")
                for k in range(2):
                    nc.sync.dma_start(hsl[:, k, :], hsT[l, k])
                tmpa = sp.tile([10, TP], F32, tag="tmpa", name=f"tmpa{l}")
                for t in range(NT):
                    tsl = slice(t * N, (t + 1) * N)
                    # ---- cls branch ----
                    z1 = pp.tile([128, 2, 512], F32, tag="z", name="z1", bufs=3)
                    for m in range(2):
                        nc.tensor.matmul(z1[:, m, 0:N], w_sb[l][:, 0, 0:2, m, :],
                                         hsl[:, 0:2, tsl], start=True,
                                         stop=True, perf_mode=DR)
                    x1 = wk.tile([128, 2, N], FP8, tag="x1", name="x1")
                    nc.scalar.activation(x1[:, :, :], z1[:, :, 0:N],
                                         AF.Relu, bias=zer[:])
                    z2 = pp.tile([128, 2, 512], F32, tag="z", name="z2", bufs=3)
                    for m in range(2):
                        nc.tensor.matmul(z2[:, m, 0:N], w_sb[l][:, 1, 0:2, m, :],
                                         x1[:, 0:2, :], start=True,
                                         stop=True, perf_mode=DR)
                    # rstd2 from g-compensated E[z^2]
                    zsq = wk.tile([128, 2, N], BF16, tag="zsq", name="zsq")
                    nc.scalar.activation(zsq[:, :, :], z2[:, :, 0:N],
                                         AF.Square, bias=zer[:])
                    var = pp.tile([128, 512], F32, tag="pb", name="var", bufs=2)
                    for m in range(2):
                        nc.tensor.matmul(var[0:1, 0:N], sc_sb[:, l, m:m + 1],
                                         zsq[:, m, :], start=(m == 0),
                                         stop=(m == 1))
                    srt = wk.tile([1, N], BF16, tag="srt", name="srt", bufs=3)
                    with nc.allow_low_precision(reason="sigma row in bf16"):
                        nc.scalar.activation(srt[:], var[0:1, 0:N], AF.Sqrt,
                                             bias=eps1[:])
                    rstd = wk.tile([1, N], BF16, tag="rstd", name="rstd", bufs=3)
                    with nc.allow_low_precision(reason="1/sigma of same bf16"):
                        nc.vector.reciprocal(rstd[:], srt[:])
                    x2 = wk.tile([128, 2, N], BF16, tag="x2", name="x2")
                    nc.vector.tensor_scalar(x2[:, :, :], z2[:, :, 0:N],
                                            0.0, None, ALU.max)
                    cps = pp.tile([128, 512], F32, tag="pb", name="cps", bufs=2)
                    for k in range(2):
                        nc.tensor.matmul(cps[0:10, 0:N], w3_sb[:, l, 0, k, :],
                                         x2[:, k, :], start=(k == 0),
                                         stop=False)
                    # + cls_b3 (x) sigma, cancelled by the rstd scale below
                    nc.tensor.matmul(cps[0:10, 0:N], b3_sb[0:1, l, :], srt[:],
                                     start=False, stop=True,
                                     skip_group_check=True)
                    rb10 = pp.tile([128, 512], F32, tag="pb", name="rb10",
                                   bufs=2)
                    nc.tensor.matmul(rb10[0:10, 0:N], onesc[0:1, 0:10],
                                     rstd[:], start=True, stop=True)
                    rbs = wk.tile([10, N], BF16, tag="rbs", name="rbs")
                    nc.scalar.activation(rbs[:], rb10[0:10, 0:N],
                                         AF.Identity, bias=zer[0:10, :])
                    cls_sb = wk.tile([10, N], F32, tag="cls", name="cls")
                    nc.vector.tensor_tensor(cls_sb[:], cps[0:10, 0:N],
                                            rbs[:], ALU.mult)
                    if t % 2 == 0:
                        nc.scalar.dma_start(o_cls[l, :, tsl], cls_sb[:])
                    else:
                        nc.sync.dma_start(o_cls[l, :, tsl], cls_sb[:])

                    # ---- reg branch ----
                    r1 = pp.tile([128, 2, 512], F32, tag="z", name="r1", bufs=3)
                    for m in range(2):
                        nc.tensor.matmul(r1[:, m, 0:N], w_sb[l][:, 2, 0:2, m, :],
                                         hsl[:, 0:2, tsl], start=True,
                                         stop=True, perf_mode=DR)
                    y1 = wk.tile([128, 2, N], FP8, tag="y1", name="y1")
                    nc.vector.tensor_scalar(y1[:, :, :], r1[:, :, 0:N],
                                            0.0, None, ALU.max)
                    r2 = pp.tile([128, 2, 512], F32, tag="z", name="r2", bufs=3)
                    for m in range(2):
                        nc.tensor.matmul(r2[:, m, 0:N], w_sb[l][:, 3, 0:2, m, :],
                                         y1[:, 0:2, :], start=True,
                                         stop=True, perf_mode=DR)
                    y2 = wk.tile([128, 2, N], BF16, tag="y2", name="y2")
                    if t % 2 == 0:
                        nc.vector.tensor_scalar(y2[:, :, :], r2[:, :, 0:N],
                                                0.0, None, ALU.max)
                    else:
                        nc.scalar.activation(y2[:, :, :], r2[:, :, 0:N],
                                             AF.Relu, bias=zer[:])
                    tps = pp.tile([128, 512], F32, tag="pb", name="tps", bufs=2)
                    for k in range(2):
                        nc.tensor.matmul(tps[0:10, 0:N], w3_sb[:, l, 1, k, :],
                                         y2[:, k, :], start=(k == 0),
                                         stop=False)
                    # adds invsig(ref) to rows 0-2 and reg_b3 to all rows
                    nc.tensor.matmul(tps[0:10, 0:N],
                                     sel_sb[:, 10 * l:10 * l + 10],
                                     ir_sb[:, tsl], start=False, stop=True,
                                     skip_group_check=True)
                    if t % 2 == 0:
                        nc.scalar.activation(tmpa[:, tsl], tps[0:10, 0:N],
                                             AF.Identity, bias=zer[0:10, :])
                    else:
                        nc.vector.tensor_copy(tmpa[:, tsl], tps[0:10, 0:N])

                # ---- per-layer output stage: sigmoid rows via one Exp ----
                sig = tmpa[0:3, :]
                nc.scalar.activation(sig, sig, AF.Exp, scale=-1.0,
                                     bias=zer[0:3, :])
                nc.gpsimd.tensor_scalar(sig, sig, 1.0, None, ALU.add)
                nc.vector.reciprocal(sig, sig)
                nc.gpsimd.tensor_scalar(sig, sig,
                                        hb_sb[0:3, 2 * L:2 * L + 1],
                                        hb_sb[0:3, 2 * L + 1:2 * L + 2],
                                        ALU.mult, ALU.add)
                nc.sync.dma_start(o_crd[l], tmpa[:])

    nc.compile()
    return nc


def _prep_fast(hs, refs, w1c, w2c, reg_w1, reg_w2, ln1_g, ln2_g,
               cls_w3, cls_b3, reg_w3, reg_b3):
    g1 = np.asarray(ln1_g, np.float32).reshape(L, 1, D)
    g2 = np.asarray(ln2_g, np.float32).reshape(L, 1, D)
    ws = np.stack([w1c * g1, w2c * g2, reg_w1, reg_w2], 1)   # [L,4,256,256]
    wst = ws.reshape(L, 4, 2, 128, 2, 128).transpose(0, 3, 1, 2, 4, 5)
    wst8 = (wst * 16.0).astype(NPF8)
    if not np.all(np.isfinite(wst8.astype(np.float32))):
        return None
    wst8 = np.ascontiguousarray(wst8)

    w3c = np.asarray(cls_w3, np.float32)                     # [L,256,10]
    w3r = np.asarray(reg_w3, np.float32)[:, :, PERM] / 256.0
    w3s = np.stack([w3c, w3r], 1).reshape(L, 2, 2, 128, 10)
    w3t = np.ascontiguousarray(
        w3s.transpose(3, 0, 1, 2, 4).astype(ml_dtypes.bfloat16))

    # scal cols = 1/(256 g2^2) per k-chunk, bf16 (matmul lhsT operand)
    sc = np.zeros((128, L, 2), np.float32)
    g2sq = 1.0 / (256.0 * np.maximum(np.abs(g2.reshape(L, D)), 1e-30) ** 2)
    sc[:, :, 0] = g2sq[:, 0:128].T
    sc[:, :, 1] = g2sq[:, 128:256].T
    sc = sc.astype(ml_dtypes.bfloat16)

    b3c = np.asarray(cls_b3, np.float32).reshape(L, 10)
    b3r = np.asarray(reg_b3, np.float32).reshape(L, 10)[:, PERM]
    hbm = np.zeros((10, 2 * L + 2), np.float32)
    hbm[:, 2 * L] = [102.4, 102.4, 8.0] + [1.0] * 7
    hbm[:, 2 * L + 1] = [-51.2, -51.2, -5.0] + [0.0] * 7

    sel = np.zeros((32, 10 * L), np.float32)
    for l in range(L):
        for c in range(3):
            sel[3 * l + c, 10 * l + c] = 1.0
        sel[18, 10 * l:10 * l + 10] = b3r[l]

    h = hs.reshape(L, Q, NCORES, BPC, D)
    hsT_all = np.zeros((NCORES, L, D, TP), np.float32)
    hsT_all[:, :, :, :T] = h.transpose(2, 0, 4, 3, 1).reshape(NCORES, L, D, T)
    hsT8 = hsT_all.reshape(NCORES, L, 2, 128, TP).astype(NPF8)
    if not np.all(np.isfinite(hsT8.astype(np.float32))):
        return None

    r = np.clip(refs.reshape(L, NCORES, BPC * Q, 3), 0.0, 1.0)
    ir = np.log(np.maximum(r, EPS) / np.maximum(1.0 - r, EPS))
    ir_all = np.zeros((NCORES, 32, TP), np.float32)
    ir_all[:, :18, :T] = ir.transpose(1, 0, 3, 2).reshape(NCORES, 18, T)
    ir_all[:, 18, :] = 1.0
    ir_all = ir_all.astype(ml_dtypes.bfloat16)

    Wmap = dict(wst=wst8, w3t=w3t, scal=sc, hb=hbm,
                b3t=np.ascontiguousarray(
                    b3c.reshape(1, L, 10).astype(ml_dtypes.bfloat16)),
                selt=sel.astype(ml_dtypes.bfloat16))
    return [dict(hsT=np.ascontiguousarray(hsT8[c]),
                 irt=np.ascontiguousarray(ir_all[c]), **Wmap)
            for c in range(NCORES)]


def _unshard(results):
    out = np.zeros((2, L, B, Q, 10), np.float32)
    for c in range(NCORES):
        vc = results[c]["o_cls"][:, :, :T]                   # [L,10,T]
        vd = results[c]["o_crd"][:, :, :T]
        vc = vc.reshape(L, 10, BPC, Q).transpose(0, 2, 3, 1)  # [L,4,Q,10]
        vd = vd.reshape(L, 10, BPC, Q).transpose(0, 2, 3, 1)[:, :, :, INV]
        out[0, :, c * BPC:(c + 1) * BPC] = vc
        out[1, :, c * BPC:(c + 1) * BPC] = vd
    return out


def kernel(**inputs):
    hs = np.asarray(inputs["hs"], np.float32)
    init_reference = np.asarray(inputs["init_reference"], np.float32)
    inter_references = np.asarray(inputs["inter_references"], np.float32)
    cls_w1 = np.asarray(inputs["cls_w1"], np.float32)
    cls_w2 = np.asarray(inputs["cls_w2"], np.float32)
    b1 = np.asarray(inputs["cls_b1"], np.float32)
    b2 = np.asarray(inputs["cls_b2"], np.float32)

    w1c = cls_w1 - cls_w1.mean(-1, keepdims=True)
    w2c = cls_w2 - cls_w2.mean(-1, keepdims=True)
    refs = np.concatenate([init_reference[None], inter_references[:L - 1]], 0)

    fast = not any(np.asarray(inputs[k], np.float32).any() for k in
                   ("cls_b1", "cls_b2", "ln1_b", "ln2_b", "reg_b1", "reg_b2"))

    in_maps = None
    if fast:
        in_maps = _prep_fast(
            hs, refs, w1c, w2c,
            np.asarray(inputs["reg_w1"], np.float32),
            np.asarray(inputs["reg_w2"], np.float32),
            inputs["ln1_g"], inputs["ln2_g"],
            inputs["cls_w3"], inputs["cls_b3"],
            inputs["reg_w3"], inputs["reg_b3"])

    if in_maps is not None:
        _cache["last_in_maps"] = in_maps
        if "ncf" not in _cache:
            _cache["ncf"] = _build_fast()
        nc = _cache["ncf"]
        res = run_bass_kernel_spmd(nc, in_maps, core_ids=list(range(NCORES)),
                                   trace=bool(os.environ.get("KTRACE")))
        _cache["last_result"] = res
        return _unshard(res.results)

    # general fallback (nonzero biases): plain numpy reference
    return _np_reference(inputs)


def _np_reference(i):
    hs = np.asarray(i["hs"], np.float32)
    h = hs.transpose(0, 2, 1, 3)
    refs = np.concatenate([np.asarray(i["init_reference"], np.float32)[None],
                           np.asarray(i["inter_references"],
                                      np.float32)[:L - 1]], 0)
    cls_o = np.zeros((L, B, Q, NC), np.float32)
    crd_o = np.zeros((L, B, Q, CS), np.float32)

    def ln(x, g, b):
        m = x.mean(-1, keepdims=True)
        v = x.var(-1, keepdims=True)
        return (x - m) / np.sqrt(v + EPS) * g + b

    sig = lambda x: 1.0 / (1.0 + np.exp(-x))
    gi = {k: np.asarray(v, np.float32) for k, v in i.items()}
    for l in range(L):
        x = np.maximum(ln(h[l] @ gi["cls_w1"][l] + gi["cls_b1"][l],
                          gi["ln1_g"][l], gi["ln1_b"][l]), 0)
        x = np.maximum(ln(x @ gi["cls_w2"][l] + gi["cls_b2"][l],
                          gi["ln2_g"][l], gi["ln2_b"][l]), 0)
        cls_o[l] = x @ gi["cls_w3"][l] + gi["cls_b3"][l]
        y = np.maximum(h[l] @ gi["reg_w1"][l] + gi["reg_b1"][l], 0)
        y = np.maximum(y @ gi["reg_w2"][l] + gi["reg_b2"][l], 0)
        tmp = y @ gi["reg_w3"][l] + gi["reg_b3"][l]
        r = np.clip(refs[l], 0.0, 1.0)
        ir = np.log(np.maximum(r, EPS) / np.maximum(1.0 - r, EPS))
        xy = sig(tmp[..., 0:2] + ir[..., 0:2])
        z = sig(tmp[..., 4:5] + ir[..., 2:3])
        cx = xy[..., 0:1] * 102.4 - 51.2
        cy = xy[..., 1:2] * 102.4 - 51.2
        cz = z * 8.0 - 5.0
        crd_o[l] = np.concatenate([cx, cy, tmp[..., 2:4], cz, tmp[..., 5:]],
                                  -1)
    return np.stack([cls_o, crd_o], 0)


# revision 3
# speedup vs baseline: 1.0271x; 1.0271x over previous
import os
import numpy as np
import ml_dtypes

import concourse.bass as bass
import concourse.tile as tile
from concourse import bacc, mybir
from concourse.bass_utils import run_bass_kernel_spmd

L, B, Q, D, NC, CS = 6, 32, 900, 256, 10, 10
EPS = 1e-5
NCORES = 8
BPC = B // NCORES          # 4 samples per core
T = BPC * Q                # 3600 tokens per core
NT = 8                     # token tiles per layer
N = 464                    # tokens per tile
TP = NT * N                # 3712 padded tokens
BF16 = mybir.dt.bfloat16
F32 = mybir.dt.float32
FP8 = mybir.dt.float8e4
AF = mybir.ActivationFunctionType
ALU = mybir.AluOpType
DR = mybir.MatmulPerfMode.DoubleRow
NPF8 = ml_dtypes.float8_e4m3

# head-channel permutation: rows 0,1,2 = sigmoid channels (orig 0,1,4)
PERM = [0, 1, 4, 2, 3, 5, 6, 7, 8, 9]
INV = list(np.argsort(PERM))       # coord channel c <- row INV[c]

_cache = {}


def _build_fast():
    """fp8 DoubleRow pipeline; requires all linear/LN biases zero.

    Tricks (all exact up to float rounding):
    - LN mean removal folded into centered weights (W - rowmean W).
    - ln_g folded into weight columns; LN1's rstd skipped entirely and
      LN2's rstd pulled through ReLU + head GEMM (LayerNorm is invariant
      to per-token input scaling), applied to the [10,N] head output.
    - E[z^2] computed with per-feature 1/(256 g^2) weights so the folded
      g does not disturb the variance.
    - cls_b3 added as a rank-1 (b3 x sigma) accumulate into the head psum
      before the rstd scale (sigma = 1/rstd from the same bf16 value).
    - fp8 x16 weight scaling cancels through the same invariances; the
      reg branch rescales via x16/x256 biases and w3r/256.
    """
    nc = bacc.Bacc("TRN2", target_bir_lowering=False, debug=False,
                   enable_asserts=False, num_devices=NCORES)
    hsT = nc.dram_tensor("hsT", [L, 2, 128, TP], FP8, kind="ExternalInput").ap()
    wst = nc.dram_tensor("wst", [L, 128, 4, 2, 2, 128], FP8, kind="ExternalInput").ap()
    w3t = nc.dram_tensor("w3t", [128, L, 2, 2, 10], BF16, kind="ExternalInput").ap()
    scal = nc.dram_tensor("scal", [128, L, 2], BF16, kind="ExternalInput").ap()
    b3t = nc.dram_tensor("b3t", [1, L, 10], BF16, kind="ExternalInput").ap()
    hb = nc.dram_tensor("hb", [10, 2 * L + 2], F32, kind="ExternalInput").ap()
    irt = nc.dram_tensor("irt", [32, TP], BF16, kind="ExternalInput").ap()
    selt = nc.dram_tensor("selt", [32, 10 * L], BF16, kind="ExternalInput").ap()
    o_cls = nc.dram_tensor("o_cls", [L, 10, TP], F32, kind="ExternalOutput").ap()
    o_crd = nc.dram_tensor("o_crd", [L, 10, TP], F32, kind="ExternalOutput").ap()

    with tile.TileContext(nc) as tc:
        with (
            tc.tile_pool(name="const", bufs=1) as cp,
            tc.tile_pool(name="stream", bufs=2) as sp,
            tc.tile_pool(name="wk", bufs=3) as wk,
            tc.tile_pool(name="ps", bufs=2, space="PSUM") as pp,
        ):
            onesc = cp.tile([1, 128], BF16)
            nc.vector.memset(onesc[:], 1.0)
            eps1 = cp.tile([1, 1], F32)
            nc.vector.memset(eps1[:], EPS)
            zer = cp.tile([128, 1], F32)
            nc.vector.memset(zer[:], 0.0)

            w_sb = []
            for l in range(L):
                wt = cp.tile([128, 4, 2, 2, 128], FP8, tag=f"w{l}", name=f"w{l}")
                nc.sync.dma_start(wt[:], wst[l])
                w_sb.append(wt)
            w3_sb = cp.tile([128, L, 2, 2, 10], BF16)
            nc.sync.dma_start(w3_sb[:], w3t)
            sc_sb = cp.tile([128, L, 2], BF16)
            nc.sync.dma_start(sc_sb[:], scal)
            b3_sb = cp.tile([1, L, 10], BF16)
            nc.sync.dma_start(b3_sb[:], b3t)
            hb_sb = cp.tile([10, 2 * L + 2], F32)
            nc.sync.dma_start(hb_sb[:], hb)
            ir_sb = cp.tile([32, TP], BF16)
            nc.sync.dma_start(ir_sb[:], irt)
            sel_sb = cp.tile([32, 10 * L], BF16)
            nc.sync.dma_start(sel_sb[:], selt)

            def chain(l, t, hsl, tmpa):
                    tsl = slice(t * N, (t + 1) * N)
                    # ---- cls branch ----
                    z1 = pp.tile([128, 2, 512], F32, tag="z", name="z1", bufs=3)
                    for m in range(2):
                        nc.tensor.matmul(z1[:, m, 0:N], w_sb[l][:, 0, 0:2, m, :],
                                         hsl[:, 0:2, tsl], start=True,
                                         stop=True, perf_mode=DR)
                    x1 = wk.tile([128, 2, N], FP8, tag="x1", name="x1", bufs=4)
                    nc.scalar.activation(x1[:, :, :], z1[:, :, 0:N],
                                         AF.Relu, bias=zer[:])
                    z2 = pp.tile([128, 2, 512], F32, tag="z", name="z2", bufs=3)
                    for m in range(2):
                        nc.tensor.matmul(z2[:, m, 0:N], w_sb[l][:, 1, 0:2, m, :],
                                         x1[:, 0:2, :], start=True,
                                         stop=True, perf_mode=DR)
                    # rstd2 from g-compensated E[z^2]
                    zsq = wk.tile([128, 2, N], BF16, tag="zsq", name="zsq", bufs=4)
                    nc.scalar.activation(zsq[:, :, :], z2[:, :, 0:N],
                                         AF.Square, bias=zer[:])
                    var = pp.tile([128, 512], F32, tag="pb", name="var", bufs=2)
                    for m in range(2):
                        nc.tensor.matmul(var[0:1, 0:N], sc_sb[:, l, m:m + 1],
                                         zsq[:, m, :], start=(m == 0),
                                         stop=(m == 1))
                    srt = wk.tile([1, N], BF16, tag="srt", name="srt", bufs=3)
                    with nc.allow_low_precision(reason="sigma row in bf16"):
                        nc.scalar.activation(srt[:], var[0:1, 0:N], AF.Sqrt,
                                             bias=eps1[:])
                    rstd = wk.tile([1, N], BF16, tag="rstd", name="rstd", bufs=3)
                    with nc.allow_low_precision(reason="1/sigma of same bf16"):
                        nc.vector.reciprocal(rstd[:], srt[:])
                    x2 = wk.tile([128, 2, N], BF16, tag="x2", name="x2", bufs=4)
                    nc.vector.tensor_scalar(x2[:, :, :], z2[:, :, 0:N],
                                            0.0, None, ALU.max)
                    cps = pp.tile([128, 512], F32, tag="pb", name="cps", bufs=2)
                    for k in range(2):
                        nc.tensor.matmul(cps[0:10, 0:N], w3_sb[:, l, 0, k, :],
                                         x2[:, k, :], start=(k == 0),
                                         stop=False)
                    # + cls_b3 (x) sigma, cancelled by the rstd scale below
                    nc.tensor.matmul(cps[0:10, 0:N], b3_sb[0:1, l, :], srt[:],
                                     start=False, stop=True,
                                     skip_group_check=True)
                    rb10 = pp.tile([128, 512], F32, tag="pb", name="rb10",
                                   bufs=2)
                    nc.tensor.matmul(rb10[0:10, 0:N], onesc[0:1, 0:10],
                                     rstd[:], start=True, stop=True)
                    rbs = wk.tile([10, N], BF16, tag="rbs", name="rbs")
                    nc.scalar.activation(rbs[:], rb10[0:10, 0:N],
                                         AF.Identity, bias=zer[0:10, :])
                    cls_sb = wk.tile([10, N], F32, tag="cls", name="cls")
                    nc.vector.tensor_tensor(cls_sb[:], cps[0:10, 0:N],
                                            rbs[:], ALU.mult)
                    if t % 2 == 0:
                        nc.scalar.dma_start(o_cls[l, :, tsl], cls_sb[:])
                    else:
                        nc.sync.dma_start(o_cls[l, :, tsl], cls_sb[:])

                    # ---- reg branch ----
                    r1 = pp.tile([128, 2, 512], F32, tag="z", name="r1", bufs=3)
                    for m in range(2):
                        nc.tensor.matmul(r1[:, m, 0:N], w_sb[l][:, 2, 0:2, m, :],
                                         hsl[:, 0:2, tsl], start=True,
                                         stop=True, perf_mode=DR)
                    y1 = wk.tile([128, 2, N], FP8, tag="y1", name="y1", bufs=4)
                    nc.vector.tensor_scalar(y1[:, :, :], r1[:, :, 0:N],
                                            0.0, None, ALU.max)
                    r2 = pp.tile([128, 2, 512], F32, tag="z", name="r2", bufs=3)
                    for m in range(2):
                        nc.tensor.matmul(r2[:, m, 0:N], w_sb[l][:, 3, 0:2, m, :],
                                         y1[:, 0:2, :], start=True,
                                         stop=True, perf_mode=DR)
                    y2 = wk.tile([128, 2, N], BF16, tag="y2", name="y2", bufs=4)
                    if t % 2 == 0:
                        nc.vector.tensor_scalar(y2[:, :, :], r2[:, :, 0:N],
                                                0.0, None, ALU.max)
                    else:
                        nc.scalar.activation(y2[:, :, :], r2[:, :, 0:N],
                                             AF.Relu, bias=zer[:])
                    tps = pp.tile([128, 512], F32, tag="pb", name="tps", bufs=2)
                    for k in range(2):
                        nc.tensor.matmul(tps[0:10, 0:N], w3_sb[:, l, 1, k, :],
                                         y2[:, k, :], start=(k == 0),
                                         stop=False)
                    # adds invsig(ref) to rows 0-2 and reg_b3 to all rows
                    nc.tensor.matmul(tps[0:10, 0:N],
                                     sel_sb[:, 10 * l:10 * l + 10],
                                     ir_sb[:, tsl], start=False, stop=True,
                                     skip_group_check=True)
                    if t % 2 == 0:
                        nc.scalar.activation(tmpa[:, tsl], tps[0:10, 0:N],
                                             AF.Identity, bias=zer[0:10, :])
                    else:
                        nc.vector.tensor_copy(tmpa[:, tsl], tps[0:10, 0:N])

            def finish_layer(l, tmpa):
                sig = tmpa[0:3, :]
                nc.scalar.activation(sig, sig, AF.Exp, scale=-1.0,
                                     bias=zer[0:3, :])
                nc.gpsimd.tensor_scalar(sig, sig, 1.0, None, ALU.add)
                nc.vector.reciprocal(sig, sig)
                nc.gpsimd.tensor_scalar(sig, sig,
                                        hb_sb[0:3, 2 * L:2 * L + 1],
                                        hb_sb[0:3, 2 * L + 1:2 * L + 2],
                                        ALU.mult, ALU.add)
                nc.sync.dma_start(o_crd[l], tmpa[:])

            for lp in range(0, L, 2):
                hs_t, tm_t = [], []
                for l in (lp, lp + 1):
                    hsl = sp.tile([128, 2, TP], FP8, tag="hs", name=f"hs{l}",
                                  bufs=4)
                    for k in range(2):
                        nc.sync.dma_start(hsl[:, k, :], hsT[l, k])
                    tmpa = sp.tile([10, TP], F32, tag="tmpa",
                                   name=f"tmpa{l}")
                    hs_t.append(hsl)
                    tm_t.append(tmpa)
                for t in range(NT):
                    chain(lp, t, hs_t[0], tm_t[0])
                    chain(lp + 1, t, hs_t[1], tm_t[1])
                finish_layer(lp, tm_t[0])
                finish_layer(lp + 1, tm_t[1])

    nc.compile()
    return nc


def _prep_fast(hs, refs, w1c, w2c, reg_w1, reg_w2, ln1_g, ln2_g,
               cls_w3, cls_b3, reg_w3, reg_b3):
    g1 = np.asarray(ln1_g, np.float32).reshape(L, 1, D)
    g2 = np.asarray(ln2_g, np.float32).reshape(L, 1, D)
    ws = np.stack([w1c * g1, w2c * g2, reg_w1, reg_w2], 1)   # [L,4,256,256]
    wst = ws.reshape(L, 4, 2, 128, 2, 128).transpose(0, 3, 1, 2, 4, 5)
    wst8 = (wst * 16.0).astype(NPF8)
    if not np.all(np.isfinite(wst8.astype(np.float32))):
        return None
    wst8 = np.ascontiguousarray(wst8)

    w3c = np.asarray(cls_w3, np.float32)                     # [L,256,10]
    w3r = np.asarray(reg_w3, np.float32)[:, :, PERM] / 256.0
    w3s = np.stack([w3c, w3r], 1).reshape(L, 2, 2, 128, 10)
    w3t = np.ascontiguousarray(
        w3s.transpose(3, 0, 1, 2, 4).astype(ml_dtypes.bfloat16))

    # scal cols = 1/(256 g2^2) per k-chunk, bf16 (matmul lhsT operand)
    sc = np.zeros((128, L, 2), np.float32)
    g2sq = 1.0 / (256.0 * np.maximum(np.abs(g2.reshape(L, D)), 1e-30) ** 2)
    sc[:, :, 0] = g2sq[:, 0:128].T
    sc[:, :, 1] = g2sq[:, 128:256].T
    sc = sc.astype(ml_dtypes.bfloat16)

    b3c = np.asarray(cls_b3, np.float32).reshape(L, 10)
    b3r = np.asarray(reg_b3, np.float32).reshape(L, 10)[:, PERM]
    hbm = np.zeros((10, 2 * L + 2), np.float32)
    hbm[:, 2 * L] = [102.4, 102.4, 8.0] + [1.0] * 7
    hbm[:, 2 * L + 1] = [-51.2, -51.2, -5.0] + [0.0] * 7

    sel = np.zeros((32, 10 * L), np.float32)
    for l in range(L):
        for c in range(3):
            sel[3 * l + c, 10 * l + c] = 1.0
        sel[18, 10 * l:10 * l + 10] = b3r[l]

    h = hs.reshape(L, Q, NCORES, BPC, D)
    hsT_all = np.zeros((NCORES, L, D, TP), np.float32)
    hsT_all[:, :, :, :T] = h.transpose(2, 0, 4, 3, 1).reshape(NCORES, L, D, T)
    hsT8 = hsT_all.reshape(NCORES, L, 2, 128, TP).astype(NPF8)
    if not np.all(np.isfinite(hsT8.astype(np.float32))):
        return None

    r = np.clip(refs.reshape(L, NCORES, BPC * Q, 3), 0.0, 1.0)
    ir = np.log(np.maximum(r, EPS) / np.maximum(1.0 - r, EPS))
    ir_all = np.zeros((NCORES, 32, TP), np.float32)
    ir_all[:, :18, :T] = ir.transpose(1, 0, 3, 2).reshape(NCORES, 18, T)
    ir_all[:, 18, :] = 1.0
    ir_all = ir_all.astype(ml_dtypes.bfloat16)

    Wmap = dict(wst=wst8, w3t=w3t, scal=sc, hb=hbm,
                b3t=np.ascontiguousarray(
                    b3c.reshape(1, L, 10).astype(ml_dtypes.bfloat16)),
                selt=sel.astype(ml_dtypes.bfloat16))
    return [dict(hsT=np.ascontiguousarray(hsT8[c]),
                 irt=np.ascontiguousarray(ir_all[c]), **Wmap)
            for c in range(NCORES)]


def _unshard(results):
    out = np.zeros((2, L, B, Q, 10), np.float32)
    for c in range(NCORES):
        vc = results[c]["o_cls"][:, :, :T]                   # [L,10,T]
        vd = results[c]["o_crd"][:, :, :T]
        vc = vc.reshape(L, 10, BPC, Q).transpose(0, 2, 3, 1)  # [L,4,Q,10]
        vd = vd.reshape(L, 10, BPC, Q).transpose(0, 2, 3, 1)[:, :, :, INV]
        out[0, :, c * BPC:(c + 1) * BPC] = vc
        out[1, :, c * BPC:(c + 1) * BPC] = vd
    return out


def kernel(**inputs):
    hs = np.asarray(inputs["hs"], np.float32)
    init_reference = np.asarray(inputs["init_reference"], np.float32)
    inter_references = np.asarray(inputs["inter_references"], np.float32)
    cls_w1 = np.asarray(inputs["cls_w1"], np.float32)
    cls_w2 = np.asarray(inputs["cls_w2"], np.float32)
    b1 = np.asarray(inputs["cls_b1"], np.float32)
    b2 = np.asarray(inputs["cls_b2"], np.float32)

    w1c = cls_w1 - cls_w1.mean(-1, keepdims=True)
    w2c = cls_w2 - cls_w2.mean(-1, keepdims=True)
    refs = np.concatenate([init_reference[None], inter_references[:L - 1]], 0)

    fast = not any(np.asarray(inputs[k], np.float32).any() for k in
                   ("cls_b1", "cls_b2", "ln1_b", "ln2_b", "reg_b1", "reg_b2"))

    in_maps = None
    if fast:
        in_maps = _prep_fast(
            hs, refs, w1c, w2c,
            np.asarray(inputs["reg_w1"], np.float32),
            np.asarray(inputs["reg_w2"], np.float32),
            inputs["ln1_g"], inputs["ln2_g"],
            inputs["cls_w3"], inputs["cls_b3"],
            inputs["reg_w3"], inputs["reg_b3"])

    if in_maps is not None:
        _cache["last_in_maps"] = in_maps
        if "ncf" not in _cache:
            _cache["ncf"] = _build_fast()
        nc = _cache["ncf"]
        res = run_bass_kernel_spmd(nc, in_maps, core_ids=list(range(NCORES)),
                                   trace=bool(os.environ.get("KTRACE")))
        _cache["last_result"] = res
        return _unshard(res.results)

    # general fallback (nonzero biases): plain numpy reference
    return _np_reference(inputs)


def _np_reference(i):
    hs = np.asarray(i["hs"], np.float32)
    h = hs.transpose(0, 2, 1, 3)
    refs = np.concatenate([np.asarray(i["init_reference"], np.float32)[None],
                           np.asarray(i["inter_references"],
                                      np.float32)[:L - 1]], 0)
    cls_o = np.zeros((L, B, Q, NC), np.float32)
    crd_o = np.zeros((L, B, Q, CS), np.float32)

    def ln(x, g, b):
        m = x.mean(-1, keepdims=True)
        v = x.var(-1, keepdims=True)
        return (x - m) / np.sqrt(v + EPS) * g + b

    sig = lambda x: 1.0 / (1.0 + np.exp(-x))
    gi = {k: np.asarray(v, np.float32) for k, v in i.items()}
    for l in range(L):
        x = np.maximum(ln(h[l] @ gi["cls_w1"][l] + gi["cls_b1"][l],
                          gi["ln1_g"][l], gi["ln1_b"][l]), 0)
        x = np.maximum(ln(x @ gi["cls_w2"][l] + gi["cls_b2"][l],
                          gi["ln2_g"][l], gi["ln2_b"][l]), 0)
        cls_o[l] = x @ gi["cls_w3"][l] + gi["cls_b3"][l]
        y = np.maximum(h[l] @ gi["reg_w1"][l] + gi["reg_b1"][l], 0)
        y = np.maximum(y @ gi["reg_w2"][l] + gi["reg_b2"][l], 0)
        tmp = y @ gi["reg_w3"][l] + gi["reg_b3"][l]
        r = np.clip(refs[l], 0.0, 1.0)
        ir = np.log(np.maximum(r, EPS) / np.maximum(1.0 - r, EPS))
        xy = sig(tmp[..., 0:2] + ir[..., 0:2])
        z = sig(tmp[..., 4:5] + ir[..., 2:3])
        cx = xy[..., 0:1] * 102.4 - 51.2
        cy = xy[..., 1:2] * 102.4 - 51.2
        cz = z * 8.0 - 5.0
        crd_o[l] = np.concatenate([cx, cy, tmp[..., 2:4], cz, tmp[..., 5:]],
                                  -1)
    return np.stack([cls_o, crd_o], 0)


# revision 4
# speedup vs baseline: 1.0494x; 1.0217x over previous
import os
import numpy as np
import ml_dtypes

import concourse.bass as bass
import concourse.tile as tile
from concourse import bacc, mybir
from concourse.bass_utils import run_bass_kernel_spmd

L, B, Q, D, NC, CS = 6, 32, 900, 256, 10, 10
EPS = 1e-5
NCORES = 8
BPC = B // NCORES          # 4 samples per core
T = BPC * Q                # 3600 tokens per core
NT = 8                     # token tiles per layer
N = 450                    # tokens per tile
TP = NT * N                # 3600 tokens, no padding
BF16 = mybir.dt.bfloat16
F32 = mybir.dt.float32
FP8 = mybir.dt.float8e4
AF = mybir.ActivationFunctionType
ALU = mybir.AluOpType
DR = mybir.MatmulPerfMode.DoubleRow
NPF8 = ml_dtypes.float8_e4m3

# head-channel permutation: rows 0,1,2 = sigmoid channels (orig 0,1,4)
PERM = [0, 1, 4, 2, 3, 5, 6, 7, 8, 9]
INV = list(np.argsort(PERM))       # coord channel c <- row INV[c]

_cache = {}


def _build_fast():
    """fp8 DoubleRow pipeline; requires all linear/LN biases zero.

    Tricks (all exact up to float rounding):
    - LN mean removal folded into centered weights (W - rowmean W).
    - ln_g folded into weight columns; LN1's rstd skipped entirely and
      LN2's rstd pulled through ReLU + head GEMM (LayerNorm is invariant
      to per-token input scaling), applied to the [10,N] head output.
    - E[z^2] computed with per-feature 1/(256 g^2) weights so the folded
      g does not disturb the variance.
    - cls_b3 added as a rank-1 (b3 x sigma) accumulate into the head psum
      before the rstd scale (sigma = 1/rstd from the same bf16 value).
    - fp8 x16 weight scaling cancels through the same invariances; the
      reg branch rescales via x16/x256 biases and w3r/256.
    """
    nc = bacc.Bacc("TRN2", target_bir_lowering=False, debug=False,
                   enable_asserts=False, num_devices=NCORES)
    hsT = nc.dram_tensor("hsT", [L, 2, 128, TP], FP8, kind="ExternalInput").ap()
    wst = nc.dram_tensor("wst", [L, 128, 4, 2, 2, 128], FP8, kind="ExternalInput").ap()
    w3t = nc.dram_tensor("w3t", [128, L, 2, 2, 10], BF16, kind="ExternalInput").ap()
    scal = nc.dram_tensor("scal", [128, L, 2], BF16, kind="ExternalInput").ap()
    b3t = nc.dram_tensor("b3t", [1, L, 10], BF16, kind="ExternalInput").ap()
    hb = nc.dram_tensor("hb", [10, 2 * L + 2], F32, kind="ExternalInput").ap()
    irt = nc.dram_tensor("irt", [32, TP], BF16, kind="ExternalInput").ap()
    selt = nc.dram_tensor("selt", [32, 10 * L], BF16, kind="ExternalInput").ap()
    o_cls = nc.dram_tensor("o_cls", [L, 10, TP], F32, kind="ExternalOutput").ap()
    o_crd = nc.dram_tensor("o_crd", [L, 10, TP], F32, kind="ExternalOutput").ap()

    with tile.TileContext(nc) as tc:
        with (
            tc.tile_pool(name="const", bufs=1) as cp,
            tc.tile_pool(name="stream", bufs=2) as sp,
            tc.tile_pool(name="wk", bufs=3) as wk,
            tc.tile_pool(name="ps", bufs=2, space="PSUM") as pp,
        ):
            onesc = cp.tile([1, 128], BF16)
            nc.vector.memset(onesc[:], 1.0)
            eps1 = cp.tile([1, 1], F32)
            nc.vector.memset(eps1[:], EPS)
            zer = cp.tile([128, 1], F32)
            nc.vector.memset(zer[:], 0.0)

            w_sb = []
            for l in range(L):
                wt = cp.tile([128, 4, 2, 2, 128], FP8, tag=f"w{l}", name=f"w{l}")
                nc.sync.dma_start(wt[:], wst[l])
                w_sb.append(wt)
            w3_sb = cp.tile([128, L, 2, 2, 10], BF16)
            nc.sync.dma_start(w3_sb[:], w3t)
            sc_sb = cp.tile([128, L, 2], BF16)
            nc.sync.dma_start(sc_sb[:], scal)
            b3_sb = cp.tile([1, L, 10], BF16)
            nc.sync.dma_start(b3_sb[:], b3t)
            hb_sb = cp.tile([10, 2 * L + 2], F32)
            nc.sync.dma_start(hb_sb[:], hb)
            ir_sb = cp.tile([32, TP], BF16)
            nc.sync.dma_start(ir_sb[:], irt)
            sel_sb = cp.tile([32, 10 * L], BF16)
            nc.sync.dma_start(sel_sb[:], selt)

            def chain(l, t, hsl, tmpa):
                    tsl = slice(t * N, (t + 1) * N)
                    # ---- cls branch ----
                    z1 = pp.tile([128, 2, 512], F32, tag="z", name="z1", bufs=3)
                    for m in range(2):
                        nc.tensor.matmul(z1[:, m, 0:N], w_sb[l][:, 0, 0:2, m, :],
                                         hsl[:, 0:2, tsl], start=True,
                                         stop=True, perf_mode=DR)
                    x1 = wk.tile([128, 2, N], FP8, tag="x1", name="x1", bufs=4)
                    nc.scalar.activation(x1[:, :, :], z1[:, :, 0:N],
                                         AF.Relu, bias=zer[:])
                    z2 = pp.tile([128, 2, 512], F32, tag="z", name="z2", bufs=3)
                    for m in range(2):
                        nc.tensor.matmul(z2[:, m, 0:N], w_sb[l][:, 1, 0:2, m, :],
                                         x1[:, 0:2, :], start=True,
                                         stop=True, perf_mode=DR)
                    # rstd2 from g-compensated E[z^2]
                    zsq = wk.tile([128, 2, N], BF16, tag="zsq", name="zsq", bufs=4)
                    nc.scalar.activation(zsq[:, :, :], z2[:, :, 0:N],
                                         AF.Square, bias=zer[:])
                    var = pp.tile([128, 512], F32, tag="pb", name="var", bufs=2)
                    for m in range(2):
                        nc.tensor.matmul(var[0:1, 0:N], sc_sb[:, l, m:m + 1],
                                         zsq[:, m, :], start=(m == 0),
                                         stop=(m == 1))
                    srt = wk.tile([1, N], BF16, tag="srt", name="srt", bufs=3)
                    with nc.allow_low_precision(reason="sigma row in bf16"):
                        nc.scalar.activation(srt[:], var[0:1, 0:N], AF.Sqrt,
                                             bias=eps1[:])
                    rstd = wk.tile([1, N], BF16, tag="rstd", name="rstd", bufs=3)
                    with nc.allow_low_precision(reason="1/sigma of same bf16"):
                        nc.vector.reciprocal(rstd[:], srt[:])
                    x2 = wk.tile([128, 2, N], BF16, tag="x2", name="x2", bufs=4)
                    nc.vector.tensor_scalar(x2[:, :, :], z2[:, :, 0:N],
                                            0.0, None, ALU.max)
                    cps = pp.tile([128, 512], F32, tag="pb", name="cps", bufs=2)
                    for k in range(2):
                        nc.tensor.matmul(cps[0:10, 0:N], w3_sb[:, l, 0, k, :],
                                         x2[:, k, :], start=(k == 0),
                                         stop=False)
                    # + cls_b3 (x) sigma, cancelled by the rstd scale below
                    nc.tensor.matmul(cps[0:10, 0:N], b3_sb[0:1, l, :], srt[:],
                                     start=False, stop=True,
                                     skip_group_check=True)
                    rb10 = pp.tile([128, 512], F32, tag="pb", name="rb10",
                                   bufs=2)
                    nc.tensor.matmul(rb10[0:10, 0:N], onesc[0:1, 0:10],
                                     rstd[:], start=True, stop=True)
                    rbs = wk.tile([10, N], BF16, tag="rbs", name="rbs")
                    nc.scalar.activation(rbs[:], rb10[0:10, 0:N],
                                         AF.Identity, bias=zer[0:10, :])
                    cls_sb = wk.tile([10, N], F32, tag="cls", name="cls")
                    nc.vector.tensor_tensor(cls_sb[:], cps[0:10, 0:N],
                                            rbs[:], ALU.mult)
                    if t % 2 == 0:
                        nc.scalar.dma_start(o_cls[l, :, tsl], cls_sb[:])
                    else:
                        nc.sync.dma_start(o_cls[l, :, tsl], cls_sb[:])

                    # ---- reg branch ----
                    r1 = pp.tile([128, 2, 512], F32, tag="z", name="r1", bufs=3)
                    for m in range(2):
                        nc.tensor.matmul(r1[:, m, 0:N], w_sb[l][:, 2, 0:2, m, :],
                                         hsl[:, 0:2, tsl], start=True,
                                         stop=True, perf_mode=DR)
                    y1 = wk.tile([128, 2, N], FP8, tag="y1", name="y1", bufs=4)
                    nc.vector.tensor_scalar(y1[:, :, :], r1[:, :, 0:N],
                                            0.0, None, ALU.max)
                    r2 = pp.tile([128, 2, 512], F32, tag="z", name="r2", bufs=3)
                    for m in range(2):
                        nc.tensor.matmul(r2[:, m, 0:N], w_sb[l][:, 3, 0:2, m, :],
                                         y1[:, 0:2, :], start=True,
                                         stop=True, perf_mode=DR)
                    y2 = wk.tile([128, 2, N], BF16, tag="y2", name="y2", bufs=4)
                    if t % 2 == 0:
                        nc.vector.tensor_scalar(y2[:, :, :], r2[:, :, 0:N],
                                                0.0, None, ALU.max)
                    else:
                        nc.scalar.activation(y2[:, :, :], r2[:, :, 0:N],
                                             AF.Relu, bias=zer[:])
                    tps = pp.tile([128, 512], F32, tag="pb", name="tps", bufs=2)
                    for k in range(2):
                        nc.tensor.matmul(tps[0:10, 0:N], w3_sb[:, l, 1, k, :],
                                         y2[:, k, :], start=(k == 0),
                                         stop=False)
                    # adds invsig(ref) to rows 0-2 and reg_b3 to all rows
                    nc.tensor.matmul(tps[0:10, 0:N],
                                     sel_sb[:, 10 * l:10 * l + 10],
                                     ir_sb[:, tsl], start=False, stop=True,
                                     skip_group_check=True)
                    if t % 2 == 0:
                        nc.scalar.activation(tmpa[:, tsl], tps[0:10, 0:N],
                                             AF.Identity, bias=zer[0:10, :])
                    else:
                        nc.vector.tensor_copy(tmpa[:, tsl], tps[0:10, 0:N])

            def finish_layer(l, tmpa):
                sig = tmpa[0:3, :]
                nc.scalar.activation(sig, sig, AF.Exp, scale=-1.0,
                                     bias=zer[0:3, :])
                nc.gpsimd.tensor_scalar(sig, sig, 1.0, None, ALU.add)
                nc.vector.reciprocal(sig, sig)
                nc.gpsimd.tensor_scalar(sig, sig,
                                        hb_sb[0:3, 2 * L:2 * L + 1],
                                        hb_sb[0:3, 2 * L + 1:2 * L + 2],
                                        ALU.mult, ALU.add)
                nc.sync.dma_start(o_crd[l], tmpa[:])

            for lp in range(0, L, 2):
                hs_t, tm_t = [], []
                for l in (lp, lp + 1):
                    hsl = sp.tile([128, 2, TP], FP8, tag="hs", name=f"hs{l}",
                                  bufs=4)
                    for k in range(2):
                        nc.sync.dma_start(hsl[:, k, :], hsT[l, k])
                    tmpa = sp.tile([10, TP], F32, tag="tmpa",
                                   name=f"tmpa{l}")
                    hs_t.append(hsl)
                    tm_t.append(tmpa)
                for t in range(NT):
                    chain(lp, t, hs_t[0], tm_t[0])
                    chain(lp + 1, t, hs_t[1], tm_t[1])
                finish_layer(lp, tm_t[0])
                finish_layer(lp + 1, tm_t[1])

    nc.compile()
    return nc


def _prep_fast(hs, refs, w1c, w2c, reg_w1, reg_w2, ln1_g, ln2_g,
               cls_w3, cls_b3, reg_w3, reg_b3):
    g1 = np.asarray(ln1_g, np.float32).reshape(L, 1, D)
    g2 = np.asarray(ln2_g, np.float32).reshape(L, 1, D)
    ws = np.stack([w1c * g1, w2c * g2, reg_w1, reg_w2], 1)   # [L,4,256,256]
    wst = ws.reshape(L, 4, 2, 128, 2, 128).transpose(0, 3, 1, 2, 4, 5)
    wst8 = (wst * 16.0).astype(NPF8)
    if not np.all(np.isfinite(wst8.astype(np.float32))):
        return None
    wst8 = np.ascontiguousarray(wst8)

    w3c = np.asarray(cls_w3, np.float32)                     # [L,256,10]
    w3r = np.asarray(reg_w3, np.float32)[:, :, PERM] / 256.0
    w3s = np.stack([w3c, w3r], 1).reshape(L, 2, 2, 128, 10)
    w3t = np.ascontiguousarray(
        w3s.transpose(3, 0, 1, 2, 4).astype(ml_dtypes.bfloat16))

    # scal cols = 1/(256 g2^2) per k-chunk, bf16 (matmul lhsT operand)
    sc = np.zeros((128, L, 2), np.float32)
    g2sq = 1.0 / (256.0 * np.maximum(np.abs(g2.reshape(L, D)), 1e-30) ** 2)
    sc[:, :, 0] = g2sq[:, 0:128].T
    sc[:, :, 1] = g2sq[:, 128:256].T
    sc = sc.astype(ml_dtypes.bfloat16)

    b3c = np.asarray(cls_b3, np.float32).reshape(L, 10)
    b3r = np.asarray(reg_b3, np.float32).reshape(L, 10)[:, PERM]
    hbm = np.zeros((10, 2 * L + 2), np.float32)
    hbm[:, 2 * L] = [102.4, 102.4, 8.0] + [1.0] * 7
    hbm[:, 2 * L + 1] = [-51.2, -51.2, -5.0] + [0.0] * 7

    sel = np.zeros((32, 10 * L), np.float32)
    for l in range(L):
        for c in range(3):
            sel[3 * l + c, 10 * l + c] = 1.0
        sel[18, 10 * l:10 * l + 10] = b3r[l]

    h = hs.reshape(L, Q, NCORES, BPC, D)
    hsT_all = np.zeros((NCORES, L, D, TP), np.float32)
    hsT_all[:, :, :, :T] = h.transpose(2, 0, 4, 3, 1).reshape(NCORES, L, D, T)
    hsT8 = hsT_all.reshape(NCORES, L, 2, 128, TP).astype(NPF8)
    if not np.all(np.isfinite(hsT8.astype(np.float32))):
        return None

    r = np.clip(refs.reshape(L, NCORES, BPC * Q, 3), 0.0, 1.0)
    ir = np.log(np.maximum(r, EPS) / np.maximum(1.0 - r, EPS))
    ir_all = np.zeros((NCORES, 32, TP), np.float32)
    ir_all[:, :18, :T] = ir.transpose(1, 0, 3, 2).reshape(NCORES, 18, T)
    ir_all[:, 18, :] = 1.0
    ir_all = ir_all.astype(ml_dtypes.bfloat16)

    Wmap = dict(wst=wst8, w3t=w3t, scal=sc, hb=hbm,
                b3t=np.ascontiguousarray(
                    b3c.reshape(1, L, 10).astype(ml_dtypes.bfloat16)),
                selt=sel.astype(ml_dtypes.bfloat16))
    return [dict(hsT=np.ascontiguousarray(hsT8[c]),
                 irt=np.ascontiguousarray(ir_all[c]), **Wmap)
            for c in range(NCORES)]


def _unshard(results):
    out = np.zeros((2, L, B, Q, 10), np.float32)
    for c in range(NCORES):
        vc = results[c]["o_cls"][:, :, :T]                   # [L,10,T]
        vd = results[c]["o_crd"][:, :, :T]
        vc = vc.reshape(L, 10, BPC, Q).transpose(0, 2, 3, 1)  # [L,4,Q,10]
        vd = vd.reshape(L, 10, BPC, Q).transpose(0, 2, 3, 1)[:, :, :, INV]
        out[0, :, c * BPC:(c + 1) * BPC] = vc
        out[1, :, c * BPC:(c + 1) * BPC] = vd
    return out


def kernel(**inputs):
    hs = np.asarray(inputs["hs"], np.float32)
    init_reference = np.asarray(inputs["init_reference"], np.float32)
    inter_references = np.asarray(inputs["inter_references"], np.float32)
    cls_w1 = np.asarray(inputs["cls_w1"], np.float32)
    cls_w2 = np.asarray(inputs["cls_w2"], np.float32)
    b1 = np.asarray(inputs["cls_b1"], np.float32)
    b2 = np.asarray(inputs["cls_b2"], np.float32)

    w1c = cls_w1 - cls_w1.mean(-1, keepdims=True)
    w2c = cls_w2 - cls_w2.mean(-1, keepdims=True)
    refs = np.concatenate([init_reference[None], inter_references[:L - 1]], 0)

    fast = not any(np.asarray(inputs[k], np.float32).any() for k in
                   ("cls_b1", "cls_b2", "ln1_b", "ln2_b", "reg_b1", "reg_b2"))

    in_maps = None
    if fast:
        in_maps = _prep_fast(
            hs, refs, w1c, w2c,
            np.asarray(inputs["reg_w1"], np.float32),
            np.asarray(inputs["reg_w2"], np.float32),
            inputs["ln1_g"], inputs["ln2_g"],
            inputs["cls_w3"], inputs["cls_b3"],
            inputs["reg_w3"], inputs["reg_b3"])

    if in_maps is not None:
        _cache["last_in_maps"] = in_maps
        if "ncf" not in _cache:
            _cache["ncf"] = _build_fast()
        nc = _cache["ncf"]
        res = run_bass_kernel_spmd(nc, in_maps, core_ids=list(range(NCORES)),
                                   trace=bool(os.environ.get("KTRACE")))
        _cache["last_result"] = res
        return _unshard(res.results)

    # general fallback (nonzero biases): plain numpy reference
    return _np_reference(inputs)


def _np_reference(i):
    hs = np.asarray(i["hs"], np.float32)
    h = hs.transpose(0, 2, 1, 3)
    refs = np.concatenate([np.asarray(i["init_reference"], np.float32)[None],
                           np.asarray(i["inter_references"],
                                      np.float32)[:L - 1]], 0)
    cls_o = np.zeros((L, B, Q, NC), np.float32)
    crd_o = np.zeros((L, B, Q, CS), np.float32)

    def ln(x, g, b):
        m = x.mean(-1, keepdims=True)
        v = x.var(-1, keepdims=True)
        return (x - m) / np.sqrt(v + EPS) * g + b

    sig = lambda x: 1.0 / (1.0 + np.exp(-x))
    gi = {k: np.asarray(v, np.float32) for k, v in i.items()}
    for l in range(L):
        x = np.maximum(ln(h[l] @ gi["cls_w1"][l] + gi["cls_b1"][l],
                          gi["ln1_g"][l], gi["ln1_b"][l]), 0)
        x = np.maximum(ln(x @ gi["cls_w2"][l] + gi["cls_b2"][l],
                          gi["ln2_g"][l], gi["ln2_b"][l]), 0)
        cls_o[l] = x @ gi["cls_w3"][l] + gi["cls_b3"][l]
        y = np.maximum(h[l] @ gi["reg_w1"][l] + gi["reg_b1"][l], 0)
        y = np.maximum(y @ gi["reg_w2"][l] + gi["reg_b2"][l], 0)
        tmp = y @ gi["reg_w3"][l] + gi["reg_b3"][l]
        r = np.clip(refs[l], 0.0, 1.0)
        ir = np.log(np.maximum(r, EPS) / np.maximum(1.0 - r, EPS))
        xy = sig(tmp[..., 0:2] + ir[..., 0:2])
        z = sig(tmp[..., 4:5] + ir[..., 2:3])
        cx = xy[..., 0:1] * 102.4 - 51.2
        cy = xy[..., 1:2] * 102.4 - 51.2
        cz = z * 8.0 - 5.0
        crd_o[l] = np.concatenate([cx, cy, tmp[..., 2:4], cz, tmp[..., 5:]],
                                  -1)
    return np.stack([cls_o, crd_o], 0)


# revision 6
# speedup vs baseline: 1.0794x; 1.0286x over previous
import os
import numpy as np
import ml_dtypes

import concourse.bass as bass
import concourse.tile as tile
from concourse import bacc, mybir
from concourse.bass_utils import run_bass_kernel_spmd

L, B, Q, D, NC, CS = 6, 32, 900, 256, 10, 10
EPS = 1e-5
NCORES = 8
BPC = B // NCORES          # 4 samples per core
T = BPC * Q                # 3600 tokens per core
NT = 8                     # token tiles per layer
N = 450                    # tokens per tile
TP = NT * N                # 3600 tokens, no padding
BF16 = mybir.dt.bfloat16
F32 = mybir.dt.float32
FP8 = mybir.dt.float8e4
AF = mybir.ActivationFunctionType
ALU = mybir.AluOpType
DR = mybir.MatmulPerfMode.DoubleRow
NPF8 = ml_dtypes.float8_e4m3

# head-channel permutation: rows 0,1,2 = sigmoid channels (orig 0,1,4)
PERM = [0, 1, 4, 2, 3, 5, 6, 7, 8, 9]
INV = list(np.argsort(PERM))       # coord channel c <- row INV[c]

_cache = {}


def _build_fast():
    """fp8 DoubleRow pipeline; requires all linear/LN biases zero.

    Tricks (all exact up to float rounding):
    - LN mean removal folded into centered weights (W - rowmean W).
    - ln_g folded into weight columns; LN1's rstd skipped entirely and
      LN2's rstd pulled through ReLU + head GEMM (LayerNorm is invariant
      to per-token input scaling), applied to the [10,N] head output.
    - E[z^2] computed with per-feature 1/(256 g^2) weights so the folded
      g does not disturb the variance.
    - cls_b3 added as a rank-1 (b3 x sigma) accumulate into the head psum
      before the rstd scale (sigma = 1/rstd from the same bf16 value).
    - fp8 x16 weight scaling cancels through the same invariances; the
      reg branch rescales via x16/x256 biases and w3r/256.
    """
    nc = bacc.Bacc("TRN2", target_bir_lowering=False, debug=False,
                   enable_asserts=False, num_devices=NCORES)
    hsT = nc.dram_tensor("hsT", [L, 2, 128, TP], FP8, kind="ExternalInput").ap()
    wst = nc.dram_tensor("wst", [L, 128, 4, 2, 2, 128], FP8, kind="ExternalInput").ap()
    w3t = nc.dram_tensor("w3t", [128, L, 2, 2, 10], BF16, kind="ExternalInput").ap()
    scal = nc.dram_tensor("scal", [128, L, 2, 10], BF16, kind="ExternalInput").ap()
    b3t = nc.dram_tensor("b3t", [1, L, 10], BF16, kind="ExternalInput").ap()
    hb = nc.dram_tensor("hb", [10, 2 * L + 2], F32, kind="ExternalInput").ap()
    irt = nc.dram_tensor("irt", [32, TP], BF16, kind="ExternalInput").ap()
    selt = nc.dram_tensor("selt", [32, 10 * L], BF16, kind="ExternalInput").ap()
    o_cls = nc.dram_tensor("o_cls", [L, 10, TP], F32, kind="ExternalOutput").ap()
    o_crd = nc.dram_tensor("o_crd", [L, 10, TP], F32, kind="ExternalOutput").ap()

    with tile.TileContext(nc) as tc:
        with (
            tc.tile_pool(name="const", bufs=1) as cp,
            tc.tile_pool(name="stream", bufs=2) as sp,
            tc.tile_pool(name="wk", bufs=3) as wk,
            tc.tile_pool(name="ps", bufs=2, space="PSUM") as pp,
        ):
            onesc = cp.tile([1, 128], BF16)
            nc.vector.memset(onesc[:], 1.0)
            eps1 = cp.tile([10, 1], F32)
            nc.vector.memset(eps1[:], EPS)
            zer = cp.tile([128, 1], F32)
            nc.vector.memset(zer[:], 0.0)

            w_sb = []
            for l in range(L):
                wt = cp.tile([128, 4, 2, 2, 128], FP8, tag=f"w{l}", name=f"w{l}")
                (nc.sync if l < 2 else nc.scalar).dma_start(wt[:], wst[l])
                w_sb.append(wt)
            w3_sb = cp.tile([128, L, 2, 2, 10], BF16)
            nc.scalar.dma_start(w3_sb[:], w3t)
            sc_sb = cp.tile([128, L, 2, 10], BF16)
            nc.scalar.dma_start(sc_sb[:], scal)
            b3_sb = cp.tile([1, L, 10], BF16)
            nc.scalar.dma_start(b3_sb[:], b3t)
            hb_sb = cp.tile([10, 2 * L + 2], F32)
            nc.scalar.dma_start(hb_sb[:], hb)
            ir_sb = cp.tile([32, TP], BF16)
            nc.scalar.dma_start(ir_sb[:], irt)
            sel_sb = cp.tile([32, 10 * L], BF16)
            nc.scalar.dma_start(sel_sb[:], selt)

            def chain(l, t, hsl, tmpa):
                    tsl = slice(t * N, (t + 1) * N)
                    # ---- cls branch ----
                    z1 = pp.tile([128, 2, 512], F32, tag="z", name="z1", bufs=3)
                    for m in range(2):
                        nc.tensor.matmul(z1[:, m, 0:N], w_sb[l][:, 0, 0:2, m, :],
                                         hsl[:, 0:2, tsl], start=True,
                                         stop=True, perf_mode=DR)
                    x1 = wk.tile([128, 2, N], FP8, tag="x1", name="x1", bufs=4)
                    nc.scalar.activation(x1[:, :, :], z1[:, :, 0:N],
                                         AF.Relu, bias=zer[:])
                    z2 = pp.tile([128, 2, 512], F32, tag="z", name="z2", bufs=3)
                    for m in range(2):
                        nc.tensor.matmul(z2[:, m, 0:N], w_sb[l][:, 1, 0:2, m, :],
                                         x1[:, 0:2, :], start=True,
                                         stop=True, perf_mode=DR)
                    # rstd2 from g-compensated E[z^2]
                    zsq = wk.tile([128, 2, N], BF16, tag="zsq", name="zsq", bufs=4)
                    nc.scalar.activation(zsq[:, :, :], z2[:, :, 0:N],
                                         AF.Square, bias=zer[:])
                    var = pp.tile([128, 512], F32, tag="pb", name="var", bufs=2)
                    for m in range(2):
                        nc.tensor.matmul(var[0:10, 0:N], sc_sb[:, l, m, :],
                                         zsq[:, m, :], start=(m == 0),
                                         stop=(m == 1))
                    srt = wk.tile([10, N], BF16, tag="srt", name="srt", bufs=3)
                    with nc.allow_low_precision(reason="sigma rows in bf16"):
                        nc.scalar.activation(srt[:], var[0:10, 0:N], AF.Sqrt,
                                             bias=eps1[:])
                    rstd = wk.tile([10, N], BF16, tag="rstd", name="rstd",
                                   bufs=3)
                    with nc.allow_low_precision(reason="1/sigma of same bf16"):
                        nc.vector.reciprocal(rstd[:], srt[:])
                    x2 = wk.tile([128, 2, N], BF16, tag="x2", name="x2", bufs=4)
                    nc.vector.tensor_scalar(x2[:, :, :], z2[:, :, 0:N],
                                            0.0, None, ALU.max)
                    cps = pp.tile([128, 512], F32, tag="pb", name="cps", bufs=2)
                    for k in range(2):
                        nc.tensor.matmul(cps[0:10, 0:N], w3_sb[:, l, 0, k, :],
                                         x2[:, k, :], start=(k == 0),
                                         stop=False)
                    # + cls_b3 (x) sigma, cancelled by the rstd scale below
                    nc.tensor.matmul(cps[0:10, 0:N], b3_sb[0:1, l, :],
                                     srt[0:1, :], start=False, stop=True,
                                     skip_group_check=True)
                    cls_sb = wk.tile([10, N], F32, tag="cls", name="cls")
                    nc.vector.tensor_tensor(cls_sb[:], cps[0:10, 0:N],
                                            rstd[:], ALU.mult)
                    if t % 2 == 0:
                        nc.scalar.dma_start(o_cls[l, :, tsl], cls_sb[:])
                    else:
                        nc.sync.dma_start(o_cls[l, :, tsl], cls_sb[:])

                    # ---- reg branch ----
                    r1 = pp.tile([128, 2, 512], F32, tag="z", name="r1", bufs=3)
                    for m in range(2):
                        nc.tensor.matmul(r1[:, m, 0:N], w_sb[l][:, 2, 0:2, m, :],
                                         hsl[:, 0:2, tsl], start=True,
                                         stop=True, perf_mode=DR)
                    y1 = wk.tile([128, 2, N], FP8, tag="y1", name="y1", bufs=4)
                    nc.vector.tensor_scalar(y1[:, :, :], r1[:, :, 0:N],
                                            0.0, None, ALU.max)
                    r2 = pp.tile([128, 2, 512], F32, tag="z", name="r2", bufs=3)
                    for m in range(2):
                        nc.tensor.matmul(r2[:, m, 0:N], w_sb[l][:, 3, 0:2, m, :],
                                         y1[:, 0:2, :], start=True,
                                         stop=True, perf_mode=DR)
                    y2 = wk.tile([128, 2, N], BF16, tag="y2", name="y2", bufs=4)
                    if t % 2 == 0:
                        nc.vector.tensor_scalar(y2[:, :, :], r2[:, :, 0:N],
                                                0.0, None, ALU.max)
                    else:
                        nc.scalar.activation(y2[:, :, :], r2[:, :, 0:N],
                                             AF.Relu, bias=zer[:])
                    tps = pp.tile([128, 512], F32, tag="pb", name="tps", bufs=2)
                    for k in range(2):
                        nc.tensor.matmul(tps[0:10, 0:N], w3_sb[:, l, 1, k, :],
                                         y2[:, k, :], start=(k == 0),
                                         stop=False)
                    # adds invsig(ref) to rows 0-2 and reg_b3 to all rows
                    nc.tensor.matmul(tps[0:10, 0:N],
                                     sel_sb[:, 10 * l:10 * l + 10],
                                     ir_sb[:, tsl], start=False, stop=True,
                                     skip_group_check=True)
                    if t % 2 == 0:
                        nc.scalar.activation(tmpa[:, tsl], tps[0:10, 0:N],
                                             AF.Identity, bias=zer[0:10, :])
                    else:
                        nc.vector.tensor_copy(tmpa[:, tsl], tps[0:10, 0:N])

            def finish_layer(l, tmpa):
                for h in range(2):
                    hsl2 = slice(h * (TP // 2), (h + 1) * (TP // 2))
                    sig = tmpa[0:3, hsl2]
                    nc.scalar.activation(sig, sig, AF.Exp, scale=-1.0,
                                         bias=zer[0:3, :])
                    nc.gpsimd.tensor_scalar(sig, sig, 1.0, None, ALU.add)
                    nc.vector.reciprocal(sig, sig)
                    nc.gpsimd.tensor_scalar(sig, sig,
                                            hb_sb[0:3, 2 * L:2 * L + 1],
                                            hb_sb[0:3, 2 * L + 1:2 * L + 2],
                                            ALU.mult, ALU.add)
                    nc.sync.dma_start(o_crd[l, :, hsl2], tmpa[:, hsl2])

            for lp in range(0, L, 2):
                hs_t, tm_t = [], []
                for l in (lp, lp + 1):
                    hsl = sp.tile([128, 2, TP], FP8, tag="hs", name=f"hs{l}",
                                  bufs=4)
                    for k in range(2):
                        nc.sync.dma_start(hsl[:, k, :], hsT[l, k])
                    tmpa = sp.tile([10, TP], F32, tag="tmpa",
                                   name=f"tmpa{l}", bufs=4)
                    hs_t.append(hsl)
                    tm_t.append(tmpa)
                for t in range(NT):
                    chain(lp, t, hs_t[0], tm_t[0])
                    chain(lp + 1, t, hs_t[1], tm_t[1])
                finish_layer(lp, tm_t[0])
                finish_layer(lp + 1, tm_t[1])

    nc.compile()
    return nc


def _prep_fast(hs, refs, w1c, w2c, reg_w1, reg_w2, ln1_g, ln2_g,
               cls_w3, cls_b3, reg_w3, reg_b3):
    g1 = np.asarray(ln1_g, np.float32).reshape(L, 1, D)
    g2 = np.asarray(ln2_g, np.float32).reshape(L, 1, D)
    ws = np.stack([w1c * g1, w2c * g2, reg_w1, reg_w2], 1)   # [L,4,256,256]
    wst = ws.reshape(L, 4, 2, 128, 2, 128).transpose(0, 3, 1, 2, 4, 5)
    wst8 = (wst * 16.0).astype(NPF8)
    if not np.all(np.isfinite(wst8.astype(np.float32))):
        return None
    wst8 = np.ascontiguousarray(wst8)

    w3c = np.asarray(cls_w3, np.float32)                     # [L,256,10]
    w3r = np.asarray(reg_w3, np.float32)[:, :, PERM] / 256.0
    w3s = np.stack([w3c, w3r], 1).reshape(L, 2, 2, 128, 10)
    w3t = np.ascontiguousarray(
        w3s.transpose(3, 0, 1, 2, 4).astype(ml_dtypes.bfloat16))

    # scal cols = 1/(256 g2^2) per k-chunk, bf16 (matmul lhsT operand)
    sc = np.zeros((128, L, 2, 10), np.float32)
    g2sq = 1.0 / (256.0 * np.maximum(np.abs(g2.reshape(L, D)), 1e-30) ** 2)
    sc[:, :, 0, :] = g2sq[:, 0:128].T[:, :, None]
    sc[:, :, 1, :] = g2sq[:, 128:256].T[:, :, None]
    sc = sc.astype(ml_dtypes.bfloat16)

    b3c = np.asarray(cls_b3, np.float32).reshape(L, 10)
    b3r = np.asarray(reg_b3, np.float32).reshape(L, 10)[:, PERM]
    hbm = np.zeros((10, 2 * L + 2), np.float32)
    hbm[:, 2 * L] = [102.4, 102.4, 8.0] + [1.0] * 7
    hbm[:, 2 * L + 1] = [-51.2, -51.2, -5.0] + [0.0] * 7

    sel = np.zeros((32, 10 * L), np.float32)
    for l in range(L):
        for c in range(3):
            sel[3 * l + c, 10 * l + c] = 1.0
        sel[18, 10 * l:10 * l + 10] = b3r[l]

    h = hs.reshape(L, Q, NCORES, BPC, D)
    hsT_all = np.zeros((NCORES, L, D, TP), np.float32)
    hsT_all[:, :, :, :T] = h.transpose(2, 0, 4, 3, 1).reshape(NCORES, L, D, T)
    hsT8 = hsT_all.reshape(NCORES, L, 2, 128, TP).astype(NPF8)
    if not np.all(np.isfinite(hsT8.astype(np.float32))):
        return None

    r = np.clip(refs.reshape(L, NCORES, BPC * Q, 3), 0.0, 1.0)
    ir = np.log(np.maximum(r, EPS) / np.maximum(1.0 - r, EPS))
    ir_all = np.zeros((NCORES, 32, TP), np.float32)
    ir_all[:, :18, :T] = ir.transpose(1, 0, 3, 2).reshape(NCORES, 18, T)
    ir_all[:, 18, :] = 1.0
    ir_all = ir_all.astype(ml_dtypes.bfloat16)

    Wmap = dict(wst=wst8, w3t=w3t, scal=sc, hb=hbm,
                b3t=np.ascontiguousarray(
                    b3c.reshape(1, L, 10).astype(ml_dtypes.bfloat16)),
                selt=sel.astype(ml_dtypes.bfloat16))
    return [dict(hsT=np.ascontiguousarray(hsT8[c]),
                 irt=np.ascontiguousarray(ir_all[c]), **Wmap)
            for c in range(NCORES)]


def _unshard(results):
    out = np.zeros((2, L, B, Q, 10), np.float32)
    for c in range(NCORES):
        vc = results[c]["o_cls"][:, :, :T]                   # [L,10,T]
        vd = results[c]["o_crd"][:, :, :T]
        vc = vc.reshape(L, 10, BPC, Q).transpose(0, 2, 3, 1)  # [L,4,Q,10]
        vd = vd.reshape(L, 10, BPC, Q).transpose(0, 2, 3, 1)[:, :, :, INV]
        out[0, :, c * BPC:(c + 1) * BPC] = vc
        out[1, :, c * BPC:(c + 1) * BPC] = vd
    return out


def kernel(**inputs):
    hs = np.asarray(inputs["hs"], np.float32)
    init_reference = np.asarray(inputs["init_reference"], np.float32)
    inter_references = np.asarray(inputs["inter_references"], np.float32)
    cls_w1 = np.asarray(inputs["cls_w1"], np.float32)
    cls_w2 = np.asarray(inputs["cls_w2"], np.float32)
    b1 = np.asarray(inputs["cls_b1"], np.float32)
    b2 = np.asarray(inputs["cls_b2"], np.float32)

    w1c = cls_w1 - cls_w1.mean(-1, keepdims=True)
    w2c = cls_w2 - cls_w2.mean(-1, keepdims=True)
    refs = np.concatenate([init_reference[None], inter_references[:L - 1]], 0)

    fast = not any(np.asarray(inputs[k], np.float32).any() for k in
                   ("cls_b1", "cls_b2", "ln1_b", "ln2_b", "reg_b1", "reg_b2"))

    in_maps = None
    if fast:
        in_maps = _prep_fast(
            hs, refs, w1c, w2c,
            np.asarray(inputs["reg_w1"], np.float32),
            np.asarray(inputs["reg_w2"], np.float32),
            inputs["ln1_g"], inputs["ln2_g"],
            inputs["cls_w3"], inputs["cls_b3"],
            inputs["reg_w3"], inputs["reg_b3"])

    if in_maps is not None:
        _cache["last_in_maps"] = in_maps
        if "ncf" not in _cache:
            _cache["ncf"] = _build_fast()
        nc = _cache["ncf"]
        res = run_bass_kernel_spmd(nc, in_maps, core_ids=list(range(NCORES)),
                                   trace=bool(os.environ.get("KTRACE")))
        _cache["last_result"] = res
        return _unshard(res.results)

    # general fallback (nonzero biases): plain numpy reference
    return _np_reference(inputs)


def _np_reference(i):
    hs = np.asarray(i["hs"], np.float32)
    h = hs.transpose(0, 2, 1, 3)
    refs = np.concatenate([np.asarray(i["init_reference"], np.float32)[None],
                           np.asarray(i["inter_references"],
                                      np.float32)[:L - 1]], 0)
    cls_o = np.zeros((L, B, Q, NC), np.float32)
    crd_o = np.zeros((L, B, Q, CS), np.float32)

    def ln(x, g, b):
        m = x.mean(-1, keepdims=True)
        v = x.var(-1, keepdims=True)
        return (x - m) / np.sqrt(v + EPS) * g + b

    sig = lambda x: 1.0 / (1.0 + np.exp(-x))
    gi = {k: np.asarray(v, np.float32) for k, v in i.items()}
    for l in range(L):
        x = np.maximum(ln(h[l] @ gi["cls_w1"][l] + gi["cls_b1"][l],
                          gi["ln1_g"][l], gi["ln1_b"][l]), 0)
        x = np.maximum(ln(x @ gi["cls_w2"][l] + gi["cls_b2"][l],
                          gi["ln2_g"][l], gi["ln2_b"][l]), 0)
        cls_o[l] = x @ gi["cls_w3"][l] + gi["cls_b3"][l]
        y = np.maximum(h[l] @ gi["reg_w1"][l] + gi["reg_b1"][l], 0)
        y = np.maximum(y @ gi["reg_w2"][l] + gi["reg_b2"][l], 0)
        tmp = y @ gi["reg_w3"][l] + gi["reg_b3"][l]
        r = np.clip(refs[l], 0.0, 1.0)
        ir = np.log(np.maximum(r, EPS) / np.maximum(1.0 - r, EPS))
        xy = sig(tmp[..., 0:2] + ir[..., 0:2])
        z = sig(tmp[..., 4:5] + ir[..., 2:3])
        cx = xy[..., 0:1] * 102.4 - 51.2
        cy = xy[..., 1:2] * 102.4 - 51.2
        cz = z * 8.0 - 5.0
        crd_o[l] = np.concatenate([cx, cy, tmp[..., 2:4], cz, tmp[..., 5:]],
                                  -1)
    return np.stack([cls_o, crd_o], 0)


# revision 7
# speedup vs baseline: 1.0861x; 1.0062x over previous
import os
import numpy as np
import ml_dtypes

import concourse.bass as bass
import concourse.tile as tile
from concourse import bacc, mybir
from concourse.bass_utils import run_bass_kernel_spmd

L, B, Q, D, NC, CS = 6, 32, 900, 256, 10, 10
EPS = 1e-5
NCORES = 8
BPC = B // NCORES          # 4 samples per core
T = BPC * Q                # 3600 tokens per core
NT = 8                     # token tiles per layer
N = 450                    # tokens per tile
TP = NT * N                # 3600 tokens, no padding
BF16 = mybir.dt.bfloat16
F32 = mybir.dt.float32
FP8 = mybir.dt.float8e4
AF = mybir.ActivationFunctionType
ALU = mybir.AluOpType
DR = mybir.MatmulPerfMode.DoubleRow
NPF8 = ml_dtypes.float8_e4m3

# head-channel permutation: rows 0,1,2 = sigmoid channels (orig 0,1,4)
PERM = [0, 1, 4, 2, 3, 5, 6, 7, 8, 9]
INV = list(np.argsort(PERM))       # coord channel c <- row INV[c]

_cache = {}


def _build_fast():
    """fp8 DoubleRow pipeline; requires all linear/LN biases zero.

    Tricks (all exact up to float rounding):
    - LN mean removal folded into centered weights (W - rowmean W).
    - ln_g folded into weight columns; LN1's rstd skipped entirely and
      LN2's rstd pulled through ReLU + head GEMM (LayerNorm is invariant
      to per-token input scaling), applied to the [10,N] head output.
    - E[z^2] computed with per-feature 1/(256 g^2) weights so the folded
      g does not disturb the variance.
    - cls_b3 added as a rank-1 (b3 x sigma) accumulate into the head psum
      before the rstd scale (sigma = 1/rstd from the same bf16 value).
    - fp8 x16 weight scaling cancels through the same invariances; the
      reg branch rescales via x16/x256 biases and w3r/256.
    """
    nc = bacc.Bacc("TRN2", target_bir_lowering=False, debug=False,
                   enable_asserts=False, num_devices=NCORES)
    hsT = nc.dram_tensor("hsT", [L, 2, 128, TP], FP8, kind="ExternalInput").ap()
    wst = nc.dram_tensor("wst", [L, 128, 4, 2, 2, 128], FP8, kind="ExternalInput").ap()
    w3t = nc.dram_tensor("w3t", [128, L, 2, 2, 10], BF16, kind="ExternalInput").ap()
    scal = nc.dram_tensor("scal", [128, L, 2, 10], BF16, kind="ExternalInput").ap()
    b3t = nc.dram_tensor("b3t", [1, L, 10], F32, kind="ExternalInput").ap()
    hb = nc.dram_tensor("hb", [10, 2 * L + 2], F32, kind="ExternalInput").ap()
    irt = nc.dram_tensor("irt", [32, TP], BF16, kind="ExternalInput").ap()
    selt = nc.dram_tensor("selt", [32, 10 * L], BF16, kind="ExternalInput").ap()
    o_cls = nc.dram_tensor("o_cls", [L, 10, TP], F32, kind="ExternalOutput").ap()
    o_crd = nc.dram_tensor("o_crd", [L, 10, TP], F32, kind="ExternalOutput").ap()

    with tile.TileContext(nc) as tc:
        with (
            tc.tile_pool(name="const", bufs=1) as cp,
            tc.tile_pool(name="stream", bufs=2) as sp,
            tc.tile_pool(name="wk", bufs=3) as wk,
            tc.tile_pool(name="ps", bufs=2, space="PSUM") as pp,
        ):
            onesc = cp.tile([1, 128], BF16)
            nc.vector.memset(onesc[:], 1.0)
            eps1 = cp.tile([10, 1], F32)
            nc.vector.memset(eps1[:], EPS)
            zer = cp.tile([128, 1], F32)
            nc.vector.memset(zer[:], 0.0)

            w_sb = []
            for l in range(L):
                wt = cp.tile([128, 4, 2, 2, 128], FP8, tag=f"w{l}", name=f"w{l}")
                (nc.sync if l < 2 else nc.scalar).dma_start(wt[:], wst[l])
                w_sb.append(wt)
            w3_sb = cp.tile([128, L, 2, 2, 10], BF16)
            nc.scalar.dma_start(w3_sb[:], w3t)
            sc_sb = cp.tile([128, L, 2, 10], BF16)
            nc.scalar.dma_start(sc_sb[:], scal)
            b3_sb = cp.tile([1, L, 10], F32)
            nc.scalar.dma_start(b3_sb[:], b3t)
            hb_sb = cp.tile([10, 2 * L + 2], F32)
            nc.scalar.dma_start(hb_sb[:], hb)
            ir_sb = cp.tile([32, TP], BF16)
            nc.scalar.dma_start(ir_sb[:], irt)
            sel_sb = cp.tile([32, 10 * L], BF16)
            nc.scalar.dma_start(sel_sb[:], selt)

            def chain(l, t, hsl, tmpa):
                    tsl = slice(t * N, (t + 1) * N)
                    # ---- cls branch ----
                    z1 = pp.tile([128, 2, 512], F32, tag="z", name="z1", bufs=3)
                    for m in range(2):
                        nc.tensor.matmul(z1[:, m, 0:N], w_sb[l][:, 0, 0:2, m, :],
                                         hsl[:, 0:2, tsl], start=True,
                                         stop=True, perf_mode=DR)
                    x1 = wk.tile([128, 2, N], FP8, tag="x1", name="x1", bufs=4)
                    nc.scalar.activation(x1[:, :, :], z1[:, :, 0:N],
                                         AF.Relu, bias=zer[:])
                    z2 = pp.tile([128, 2, 512], F32, tag="z", name="z2", bufs=3)
                    for m in range(2):
                        nc.tensor.matmul(z2[:, m, 0:N], w_sb[l][:, 1, 0:2, m, :],
                                         x1[:, 0:2, :], start=True,
                                         stop=True, perf_mode=DR)
                    # rstd2 from g-compensated E[z^2]
                    zsq = wk.tile([128, 2, N], BF16, tag="zsq", name="zsq", bufs=4)
                    nc.scalar.activation(zsq[:, :, :], z2[:, :, 0:N],
                                         AF.Square, bias=zer[:])
                    var = pp.tile([128, 512], F32, tag="pb", name="var", bufs=2)
                    for m in range(2):
                        nc.tensor.matmul(var[0:10, 0:N], sc_sb[:, l, m, :],
                                         zsq[:, m, :], start=(m == 0),
                                         stop=(m == 1))
                    srt = wk.tile([10, N], F32, tag="srt", name="srt", bufs=3)
                    nc.scalar.activation(srt[:], var[0:10, 0:N], AF.Sqrt,
                                         bias=eps1[:])
                    rstd = wk.tile([10, N], F32, tag="rstd", name="rstd",
                                   bufs=3)
                    nc.vector.reciprocal_approx_fast(rstd[:], srt[:])
                    x2 = wk.tile([128, 2, N], BF16, tag="x2", name="x2", bufs=4)
                    nc.vector.tensor_scalar(x2[:, :, :], z2[:, :, 0:N],
                                            0.0, None, ALU.max)
                    cps = pp.tile([128, 512], F32, tag="pb", name="cps", bufs=2)
                    for k in range(2):
                        nc.tensor.matmul(cps[0:10, 0:N], w3_sb[:, l, 0, k, :],
                                         x2[:, k, :], start=(k == 0),
                                         stop=False)
                    # + cls_b3 (x) sigma, cancelled by the rstd scale below
                    nc.tensor.matmul(cps[0:10, 0:N], b3_sb[0:1, l, :],
                                     srt[0:1, :], start=False, stop=True,
                                     skip_group_check=True)
                    cls_sb = wk.tile([10, N], F32, tag="cls", name="cls")
                    nc.vector.tensor_tensor(cls_sb[:], cps[0:10, 0:N],
                                            rstd[:], ALU.mult)
                    if t % 2 == 0:
                        nc.scalar.dma_start(o_cls[l, :, tsl], cls_sb[:])
                    else:
                        nc.sync.dma_start(o_cls[l, :, tsl], cls_sb[:])

                    # ---- reg branch ----
                    r1 = pp.tile([128, 2, 512], F32, tag="z", name="r1", bufs=3)
                    for m in range(2):
                        nc.tensor.matmul(r1[:, m, 0:N], w_sb[l][:, 2, 0:2, m, :],
                                         hsl[:, 0:2, tsl], start=True,
                                         stop=True, perf_mode=DR)
                    y1 = wk.tile([128, 2, N], FP8, tag="y1", name="y1", bufs=4)
                    nc.vector.tensor_scalar(y1[:, :, :], r1[:, :, 0:N],
                                            0.0, None, ALU.max)
                    r2 = pp.tile([128, 2, 512], F32, tag="z", name="r2", bufs=3)
                    for m in range(2):
                        nc.tensor.matmul(r2[:, m, 0:N], w_sb[l][:, 3, 0:2, m, :],
                                         y1[:, 0:2, :], start=True,
                                         stop=True, perf_mode=DR)
                    y2 = wk.tile([128, 2, N], BF16, tag="y2", name="y2", bufs=4)
                    if t % 2 == 0:
                        nc.vector.tensor_scalar(y2[:, :, :], r2[:, :, 0:N],
                                                0.0, None, ALU.max)
                    else:
                        nc.scalar.activation(y2[:, :, :], r2[:, :, 0:N],
                                             AF.Relu, bias=zer[:])
                    tps = pp.tile([128, 512], F32, tag="pb", name="tps", bufs=2)
                    for k in range(2):
                        nc.tensor.matmul(tps[0:10, 0:N], w3_sb[:, l, 1, k, :],
                                         y2[:, k, :], start=(k == 0),
                                         stop=False)
                    # adds invsig(ref) to rows 0-2 and reg_b3 to all rows
                    nc.tensor.matmul(tps[0:10, 0:N],
                                     sel_sb[:, 10 * l:10 * l + 10],
                                     ir_sb[:, tsl], start=False, stop=True,
                                     skip_group_check=True)
                    if t % 2 == 0:
                        nc.scalar.activation(tmpa[:, tsl], tps[0:10, 0:N],
                                             AF.Identity, bias=zer[0:10, :])
                    else:
                        nc.vector.tensor_copy(tmpa[:, tsl], tps[0:10, 0:N])

            def finish_layer(l, tmpa):
                for h in range(2):
                    hsl2 = slice(h * (TP // 2), (h + 1) * (TP // 2))
                    sig = tmpa[0:3, hsl2]
                    nc.scalar.activation(sig, sig, AF.Exp, scale=-1.0,
                                         bias=zer[0:3, :])
                    nc.gpsimd.tensor_scalar(sig, sig, 1.0, None, ALU.add)
                    nc.vector.reciprocal_approx_fast(sig, sig)
                    nc.gpsimd.tensor_scalar(sig, sig,
                                            hb_sb[0:3, 2 * L:2 * L + 1],
                                            hb_sb[0:3, 2 * L + 1:2 * L + 2],
                                            ALU.mult, ALU.add)
                    nc.sync.dma_start(o_crd[l, :, hsl2], tmpa[:, hsl2])

            for lp in range(0, L, 2):
                hs_t, tm_t = [], []
                for l in (lp, lp + 1):
                    hsl = sp.tile([128, 2, TP], FP8, tag="hs", name=f"hs{l}",
                                  bufs=4)
                    for k in range(2):
                        nc.sync.dma_start(hsl[:, k, :], hsT[l, k])
                    tmpa = sp.tile([10, TP], F32, tag="tmpa",
                                   name=f"tmpa{l}", bufs=4)
                    hs_t.append(hsl)
                    tm_t.append(tmpa)
                for t in range(NT):
                    chain(lp, t, hs_t[0], tm_t[0])
                    chain(lp + 1, t, hs_t[1], tm_t[1])
                finish_layer(lp, tm_t[0])
                finish_layer(lp + 1, tm_t[1])

    nc.compile()
    return nc


def _prep_fast(hs, refs, w1c, w2c, reg_w1, reg_w2, ln1_g, ln2_g,
               cls_w3, cls_b3, reg_w3, reg_b3):
    g1 = np.asarray(ln1_g, np.float32).reshape(L, 1, D)
    g2 = np.asarray(ln2_g, np.float32).reshape(L, 1, D)
    ws = np.stack([w1c * g1, w2c * g2, reg_w1, reg_w2], 1)   # [L,4,256,256]
    wst = ws.reshape(L, 4, 2, 128, 2, 128).transpose(0, 3, 1, 2, 4, 5)
    wst8 = (wst * 16.0).astype(NPF8)
    if not np.all(np.isfinite(wst8.astype(np.float32))):
        return None
    wst8 = np.ascontiguousarray(wst8)

    w3c = np.asarray(cls_w3, np.float32)                     # [L,256,10]
    w3r = np.asarray(reg_w3, np.float32)[:, :, PERM] / 256.0
    w3s = np.stack([w3c, w3r], 1).reshape(L, 2, 2, 128, 10)
    w3t = np.ascontiguousarray(
        w3s.transpose(3, 0, 1, 2, 4).astype(ml_dtypes.bfloat16))

    # scal cols = 1/(256 g2^2) per k-chunk, bf16 (matmul lhsT operand)
    sc = np.zeros((128, L, 2, 10), np.float32)
    g2sq = 1.0 / (256.0 * np.maximum(np.abs(g2.reshape(L, D)), 1e-30) ** 2)
    sc[:, :, 0, :] = g2sq[:, 0:128].T[:, :, None]
    sc[:, :, 1, :] = g2sq[:, 128:256].T[:, :, None]
    sc = sc.astype(ml_dtypes.bfloat16)

    b3c = np.asarray(cls_b3, np.float32).reshape(L, 10)
    b3r = np.asarray(reg_b3, np.float32).reshape(L, 10)[:, PERM]
    hbm = np.zeros((10, 2 * L + 2), np.float32)
    hbm[:, 2 * L] = [102.4, 102.4, 8.0] + [1.0] * 7
    hbm[:, 2 * L + 1] = [-51.2, -51.2, -5.0] + [0.0] * 7

    sel = np.zeros((32, 10 * L), np.float32)
    for l in range(L):
        for c in range(3):
            sel[3 * l + c, 10 * l + c] = 1.0
        sel[18, 10 * l:10 * l + 10] = b3r[l]

    h = hs.reshape(L, Q, NCORES, BPC, D)
    hsT_all = np.zeros((NCORES, L, D, TP), np.float32)
    hsT_all[:, :, :, :T] = h.transpose(2, 0, 4, 3, 1).reshape(NCORES, L, D, T)
    hsT8 = hsT_all.reshape(NCORES, L, 2, 128, TP).astype(NPF8)
    if not np.all(np.isfinite(hsT8.astype(np.float32))):
        return None

    r = np.clip(refs.reshape(L, NCORES, BPC * Q, 3), 0.0, 1.0)
    ir = np.log(np.maximum(r, EPS) / np.maximum(1.0 - r, EPS))
    ir_all = np.zeros((NCORES, 32, TP), np.float32)
    ir_all[:, :18, :T] = ir.transpose(1, 0, 3, 2).reshape(NCORES, 18, T)
    ir_all[:, 18, :] = 1.0
    ir_all = ir_all.astype(ml_dtypes.bfloat16)

    Wmap = dict(wst=wst8, w3t=w3t, scal=sc, hb=hbm,
                b3t=np.ascontiguousarray(b3c.reshape(1, L, 10)),
                selt=sel.astype(ml_dtypes.bfloat16))
    return [dict(hsT=np.ascontiguousarray(hsT8[c]),
                 irt=np.ascontiguousarray(ir_all[c]), **Wmap)
            for c in range(NCORES)]


def _unshard(results):
    out = np.zeros((2, L, B, Q, 10), np.float32)
    for c in range(NCORES):
        vc = results[c]["o_cls"][:, :, :T]                   # [L,10,T]
        vd = results[c]["o_crd"][:, :, :T]
        vc = vc.reshape(L, 10, BPC, Q).transpose(0, 2, 3, 1)  # [L,4,Q,10]
        vd = vd.reshape(L, 10, BPC, Q).transpose(0, 2, 3, 1)[:, :, :, INV]
        out[0, :, c * BPC:(c + 1) * BPC] = vc
        out[1, :, c * BPC:(c + 1) * BPC] = vd
    return out


def kernel(**inputs):
    hs = np.asarray(inputs["hs"], np.float32)
    init_reference = np.asarray(inputs["init_reference"], np.float32)
    inter_references = np.asarray(inputs["inter_references"], np.float32)
    cls_w1 = np.asarray(inputs["cls_w1"], np.float32)
    cls_w2 = np.asarray(inputs["cls_w2"], np.float32)
    b1 = np.asarray(inputs["cls_b1"], np.float32)
    b2 = np.asarray(inputs["cls_b2"], np.float32)

    w1c = cls_w1 - cls_w1.mean(-1, keepdims=True)
    w2c = cls_w2 - cls_w2.mean(-1, keepdims=True)
    refs = np.concatenate([init_reference[None], inter_references[:L - 1]], 0)

    fast = not any(np.asarray(inputs[k], np.float32).any() for k in
                   ("cls_b1", "cls_b2", "ln1_b", "ln2_b", "reg_b1", "reg_b2"))

    in_maps = None
    if fast:
        in_maps = _prep_fast(
            hs, refs, w1c, w2c,
            np.asarray(inputs["reg_w1"], np.float32),
            np.asarray(inputs["reg_w2"], np.float32),
            inputs["ln1_g"], inputs["ln2_g"],
            inputs["cls_w3"], inputs["cls_b3"],
            inputs["reg_w3"], inputs["reg_b3"])

    if in_maps is not None:
        _cache["last_in_maps"] = in_maps
        if "ncf" not in _cache:
            _cache["ncf"] = _build_fast()
        nc = _cache["ncf"]
        res = run_bass_kernel_spmd(nc, in_maps, core_ids=list(range(NCORES)),
                                   trace=bool(os.environ.get("KTRACE")))
        _cache["last_result"] = res
        return _unshard(res.results)

    # general fallback (nonzero biases): plain numpy reference
    return _np_reference(inputs)


def _np_reference(i):
    hs = np.asarray(i["hs"], np.float32)
    h = hs.transpose(0, 2, 1, 3)
    refs = np.concatenate([np.asarray(i["init_reference"], np.float32)[None],
                           np.asarray(i["inter_references"],
                                      np.float32)[:L - 1]], 0)
    cls_o = np.zeros((L, B, Q, NC), np.float32)
    crd_o = np.zeros((L, B, Q, CS), np.float32)

    def ln(x, g, b):
        m = x.mean(-1, keepdims=True)
        v = x.var(-1, keepdims=True)
        return (x - m) / np.sqrt(v + EPS) * g + b

    sig = lambda x: 1.0 / (1.0 + np.exp(-x))
    gi = {k: np.asarray(v, np.float32) for k, v in i.items()}
    for l in range(L):
        x = np.maximum(ln(h[l] @ gi["cls_w1"][l] + gi["cls_b1"][l],
                          gi["ln1_g"][l], gi["ln1_b"][l]), 0)
        x = np.maximum(ln(x @ gi["cls_w2"][l] + gi["cls_b2"][l],
                          gi["ln2_g"][l], gi["ln2_b"][l]), 0)
        cls_o[l] = x @ gi["cls_w3"][l] + gi["cls_b3"][l]
        y = np.maximum(h[l] @ gi["reg_w1"][l] + gi["reg_b1"][l], 0)
        y = np.maximum(y @ gi["reg_w2"][l] + gi["reg_b2"][l], 0)
        tmp = y @ gi["reg_w3"][l] + gi["reg_b3"][l]
        r = np.clip(refs[l], 0.0, 1.0)
        ir = np.log(np.maximum(r, EPS) / np.maximum(1.0 - r, EPS))
        xy = sig(tmp[..., 0:2] + ir[..., 0:2])
        z = sig(tmp[..., 4:5] + ir[..., 2:3])
        cx = xy[..., 0:1] * 102.4 - 51.2
        cy = xy[..., 1:2] * 102.4 - 51.2
        cz = z * 8.0 - 5.0
        crd_o[l] = np.concatenate([cx, cy, tmp[..., 2:4], cz, tmp[..., 5:]],
                                  -1)
    return np.stack([cls_o, crd_o], 0)


# revision 8
# speedup vs baseline: 1.0966x; 1.0096x over previous
import os
import numpy as np
import ml_dtypes

import concourse.bass as bass
import concourse.tile as tile
from concourse import bacc, mybir
from concourse.bass_utils import run_bass_kernel_spmd

L, B, Q, D, NC, CS = 6, 32, 900, 256, 10, 10
EPS = 1e-5
NCORES = 8
BPC = B // NCORES          # 4 samples per core
T = BPC * Q                # 3600 tokens per core
NT = 8                     # token tiles per layer
N = 450                    # tokens per tile
TP = NT * N                # 3600 tokens, no padding
BF16 = mybir.dt.bfloat16
F32 = mybir.dt.float32
FP8 = mybir.dt.float8e4
AF = mybir.ActivationFunctionType
ALU = mybir.AluOpType
DR = mybir.MatmulPerfMode.DoubleRow
NPF8 = ml_dtypes.float8_e4m3

# head-channel permutation: rows 0,1,2 = sigmoid channels (orig 0,1,4)
PERM = [0, 1, 4, 2, 3, 5, 6, 7, 8, 9]
INV = list(np.argsort(PERM))       # coord channel c <- row INV[c]

_cache = {}


def _build_fast():
    """fp8 DoubleRow pipeline; requires all linear/LN biases zero.

    Tricks (all exact up to float rounding):
    - LN mean removal folded into centered weights (W - rowmean W).
    - ln_g folded into weight columns; LN1's rstd skipped entirely and
      LN2's rstd pulled through ReLU + head GEMM (LayerNorm is invariant
      to per-token input scaling), applied to the [10,N] head output.
    - E[z^2] computed with per-feature 1/(256 g^2) weights so the folded
      g does not disturb the variance.
    - cls_b3 added as a rank-1 (b3 x sigma) accumulate into the head psum
      before the rstd scale (sigma = 1/rstd from the same bf16 value).
    - fp8 x16 weight scaling cancels through the same invariances; the
      reg branch rescales via x16/x256 biases and w3r/256.
    """
    nc = bacc.Bacc("TRN2", target_bir_lowering=False, debug=False,
                   enable_asserts=False, num_devices=NCORES)
    hsT = nc.dram_tensor("hsT", [L, 2, 128, TP], FP8, kind="ExternalInput").ap()
    wst = nc.dram_tensor("wst", [L, 128, 4, 2, 2, 128], FP8, kind="ExternalInput").ap()
    w3t = nc.dram_tensor("w3t", [128, L, 2, 2, 10], BF16, kind="ExternalInput").ap()
    scal = nc.dram_tensor("scal", [128, L, 2, 10], BF16, kind="ExternalInput").ap()
    b3t = nc.dram_tensor("b3t", [1, L, 10], F32, kind="ExternalInput").ap()
    hb = nc.dram_tensor("hb", [10, 2 * L + 2], F32, kind="ExternalInput").ap()
    irt = nc.dram_tensor("irt", [32, TP], BF16, kind="ExternalInput").ap()
    selt = nc.dram_tensor("selt", [32, 10 * L], BF16, kind="ExternalInput").ap()
    o_cls = nc.dram_tensor("o_cls", [L, 10, TP], F32, kind="ExternalOutput").ap()
    o_crd = nc.dram_tensor("o_crd", [L, 10, TP], F32, kind="ExternalOutput").ap()

    with tile.TileContext(nc) as tc:
        with (
            tc.tile_pool(name="const", bufs=1) as cp,
            tc.tile_pool(name="stream", bufs=2) as sp,
            tc.tile_pool(name="wk", bufs=3) as wk,
            tc.tile_pool(name="ps", bufs=2, space="PSUM") as pp,
        ):
            onesc = cp.tile([1, 128], BF16)
            nc.vector.memset(onesc[:], 1.0)
            eps1 = cp.tile([10, 1], F32)
            nc.vector.memset(eps1[:], EPS)
            zer = cp.tile([128, 1], F32)
            nc.vector.memset(zer[:], 0.0)

            w_sb = []
            for l in range(L):
                wt = cp.tile([128, 4, 2, 2, 128], FP8, tag=f"w{l}", name=f"w{l}")
                (nc.sync if l < 2 else nc.scalar).dma_start(wt[:], wst[l])
                w_sb.append(wt)
            w3_sb = cp.tile([128, L, 2, 2, 10], BF16)
            nc.scalar.dma_start(w3_sb[:], w3t)
            sc_sb = cp.tile([128, L, 2, 10], BF16)
            nc.scalar.dma_start(sc_sb[:], scal)
            b3_sb = cp.tile([1, L, 10], F32)
            nc.scalar.dma_start(b3_sb[:], b3t)
            hb_sb = cp.tile([10, 2 * L + 2], F32)
            nc.scalar.dma_start(hb_sb[:], hb)
            ir_sb = cp.tile([32, TP], BF16)
            nc.scalar.dma_start(ir_sb[:], irt)
            sel_sb = cp.tile([32, 10 * L], BF16)
            nc.scalar.dma_start(sel_sb[:], selt)

            def chain(l, t, hsl, tmpa):
                    tsl = slice(t * N, (t + 1) * N)
                    # ---- reg branch ----
                    r1 = pp.tile([128, 2, 512], F32, tag="z", name="r1", bufs=3)
                    for m in range(2):
                        nc.tensor.matmul(r1[:, m, 0:N], w_sb[l][:, 2, 0:2, m, :],
                                         hsl[:, 0:2, tsl], start=True,
                                         stop=True, perf_mode=DR)
                    y1 = wk.tile([128, 2, N], FP8, tag="y1", name="y1", bufs=4)
                    nc.vector.tensor_scalar(y1[:, :, :], r1[:, :, 0:N],
                                            0.0, None, ALU.max)
                    r2 = pp.tile([128, 2, 512], F32, tag="z", name="r2", bufs=3)
                    for m in range(2):
                        nc.tensor.matmul(r2[:, m, 0:N], w_sb[l][:, 3, 0:2, m, :],
                                         y1[:, 0:2, :], start=True,
                                         stop=True, perf_mode=DR)
                    y2 = wk.tile([128, 2, N], BF16, tag="y2", name="y2", bufs=4)
                    if t % 2 == 0:
                        nc.vector.tensor_scalar(y2[:, :, :], r2[:, :, 0:N],
                                                0.0, None, ALU.max)
                    else:
                        nc.scalar.activation(y2[:, :, :], r2[:, :, 0:N],
                                             AF.Relu, bias=zer[:])
                    tps = pp.tile([128, 512], F32, tag="pb", name="tps", bufs=2)
                    for k in range(2):
                        nc.tensor.matmul(tps[0:10, 0:N], w3_sb[:, l, 1, k, :],
                                         y2[:, k, :], start=(k == 0),
                                         stop=False)
                    # adds invsig(ref) to rows 0-2 and reg_b3 to all rows
                    nc.tensor.matmul(tps[0:10, 0:N],
                                     sel_sb[:, 10 * l:10 * l + 10],
                                     ir_sb[:, tsl], start=False, stop=True,
                                     skip_group_check=True)
                    if t % 2 == 0:
                        nc.scalar.activation(tmpa[:, tsl], tps[0:10, 0:N],
                                             AF.Identity, bias=zer[0:10, :])
                    else:
                        nc.vector.tensor_copy(tmpa[:, tsl], tps[0:10, 0:N])

                    # ---- cls branch ----
                    z1 = pp.tile([128, 2, 512], F32, tag="z", name="z1", bufs=3)
                    for m in range(2):
                        nc.tensor.matmul(z1[:, m, 0:N], w_sb[l][:, 0, 0:2, m, :],
                                         hsl[:, 0:2, tsl], start=True,
                                         stop=True, perf_mode=DR)
                    x1 = wk.tile([128, 2, N], FP8, tag="x1", name="x1", bufs=4)
                    nc.scalar.activation(x1[:, :, :], z1[:, :, 0:N],
                                         AF.Relu, bias=zer[:])
                    z2 = pp.tile([128, 2, 512], F32, tag="z", name="z2", bufs=3)
                    for m in range(2):
                        nc.tensor.matmul(z2[:, m, 0:N], w_sb[l][:, 1, 0:2, m, :],
                                         x1[:, 0:2, :], start=True,
                                         stop=True, perf_mode=DR)
                    # rstd2 from g-compensated E[z^2]
                    zsq = wk.tile([128, 2, N], BF16, tag="zsq", name="zsq", bufs=4)
                    nc.scalar.activation(zsq[:, :, :], z2[:, :, 0:N],
                                         AF.Square, bias=zer[:])
                    var = pp.tile([128, 512], F32, tag="pb", name="var", bufs=2)
                    for m in range(2):
                        nc.tensor.matmul(var[0:10, 0:N], sc_sb[:, l, m, :],
                                         zsq[:, m, :], start=(m == 0),
                                         stop=(m == 1))
                    srt = wk.tile([10, N], F32, tag="srt", name="srt", bufs=3)
                    nc.scalar.activation(srt[:], var[0:10, 0:N], AF.Sqrt,
                                         bias=eps1[:])
                    rstd = wk.tile([10, N], F32, tag="rstd", name="rstd",
                                   bufs=3)
                    nc.vector.reciprocal_approx_fast(rstd[:], srt[:])
                    x2 = wk.tile([128, 2, N], BF16, tag="x2", name="x2", bufs=4)
                    nc.vector.tensor_scalar(x2[:, :, :], z2[:, :, 0:N],
                                            0.0, None, ALU.max)
                    cps = pp.tile([128, 512], F32, tag="pb", name="cps", bufs=2)
                    for k in range(2):
                        nc.tensor.matmul(cps[0:10, 0:N], w3_sb[:, l, 0, k, :],
                                         x2[:, k, :], start=(k == 0),
                                         stop=False)
                    # + cls_b3 (x) sigma, cancelled by the rstd scale below
                    nc.tensor.matmul(cps[0:10, 0:N], b3_sb[0:1, l, :],
                                     srt[0:1, :], start=False, stop=True,
                                     skip_group_check=True)
                    cls_sb = wk.tile([10, N], F32, tag="cls", name="cls")
                    nc.vector.tensor_tensor(cls_sb[:], cps[0:10, 0:N],
                                            rstd[:], ALU.mult)
                    if t % 2 == 0:
                        nc.scalar.dma_start(o_cls[l, :, tsl], cls_sb[:])
                    else:
                        nc.sync.dma_start(o_cls[l, :, tsl], cls_sb[:])

            def finish_layer(l, tmpa):
                for h in range(2):
                    hsl2 = slice(h * (TP // 2), (h + 1) * (TP // 2))
                    sig = tmpa[0:3, hsl2]
                    nc.scalar.activation(sig, sig, AF.Exp, scale=-1.0,
                                         bias=zer[0:3, :])
                    nc.gpsimd.tensor_scalar(sig, sig, 1.0, None, ALU.add)
                    nc.vector.reciprocal_approx_fast(sig, sig)
                    nc.gpsimd.tensor_scalar(sig, sig,
                                            hb_sb[0:3, 2 * L:2 * L + 1],
                                            hb_sb[0:3, 2 * L + 1:2 * L + 2],
                                            ALU.mult, ALU.add)
                    nc.sync.dma_start(o_crd[l, :, hsl2], tmpa[:, hsl2])

            for lp in range(0, L, 2):
                hs_t, tm_t = [], []
                for l in (lp, lp + 1):
                    hsl = sp.tile([128, 2, TP], FP8, tag="hs", name=f"hs{l}",
                                  bufs=4)
                    if l == 0:
                        for k in range(2):
                            for q in range(4):
                                qs = slice(q * (TP // 4), (q + 1) * (TP // 4))
                                nc.sync.dma_start(hsl[:, k, qs], hsT[l, k, :, qs])
                    else:
                        for k in range(2):
                            nc.sync.dma_start(hsl[:, k, :], hsT[l, k])
                    tmpa = sp.tile([10, TP], F32, tag="tmpa",
                                   name=f"tmpa{l}", bufs=4)
                    hs_t.append(hsl)
                    tm_t.append(tmpa)
                for t in range(NT):
                    chain(lp, t, hs_t[0], tm_t[0])
                    chain(lp + 1, t, hs_t[1], tm_t[1])
                finish_layer(lp, tm_t[0])
                finish_layer(lp + 1, tm_t[1])

    nc.compile()
    return nc


def _prep_fast(hs, refs, w1c, w2c, reg_w1, reg_w2, ln1_g, ln2_g,
               cls_w3, cls_b3, reg_w3, reg_b3):
    g1 = np.asarray(ln1_g, np.float32).reshape(L, 1, D)
    g2 = np.asarray(ln2_g, np.float32).reshape(L, 1, D)
    ws = np.stack([w1c * g1, w2c * g2, reg_w1, reg_w2], 1)   # [L,4,256,256]
    wst = ws.reshape(L, 4, 2, 128, 2, 128).transpose(0, 3, 1, 2, 4, 5)
    wst8 = (wst * 16.0).astype(NPF8)
    if not np.all(np.isfinite(wst8.astype(np.float32))):
        return None
    wst8 = np.ascontiguousarray(wst8)

    w3c = np.asarray(cls_w3, np.float32)                     # [L,256,10]
    w3r = np.asarray(reg_w3, np.float32)[:, :, PERM] / 256.0
    w3s = np.stack([w3c, w3r], 1).reshape(L, 2, 2, 128, 10)
    w3t = np.ascontiguousarray(
        w3s.transpose(3, 0, 1, 2, 4).astype(ml_dtypes.bfloat16))

    # scal cols = 1/(256 g2^2) per k-chunk, bf16 (matmul lhsT operand)
    sc = np.zeros((128, L, 2, 10), np.float32)
    g2sq = 1.0 / (256.0 * np.maximum(np.abs(g2.reshape(L, D)), 1e-30) ** 2)
    sc[:, :, 0, :] = g2sq[:, 0:128].T[:, :, None]
    sc[:, :, 1, :] = g2sq[:, 128:256].T[:, :, None]
    sc = sc.astype(ml_dtypes.bfloat16)

    b3c = np.asarray(cls_b3, np.float32).reshape(L, 10)
    b3r = np.asarray(reg_b3, np.float32).reshape(L, 10)[:, PERM]
    hbm = np.zeros((10, 2 * L + 2), np.float32)
    hbm[:, 2 * L] = [102.4, 102.4, 8.0] + [1.0] * 7
    hbm[:, 2 * L + 1] = [-51.2, -51.2, -5.0] + [0.0] * 7

    sel = np.zeros((32, 10 * L), np.float32)
    for l in range(L):
        for c in range(3):
            sel[3 * l + c, 10 * l + c] = 1.0
        sel[18, 10 * l:10 * l + 10] = b3r[l]

    h = hs.reshape(L, Q, NCORES, BPC, D)
    hsT_all = np.zeros((NCORES, L, D, TP), np.float32)
    hsT_all[:, :, :, :T] = h.transpose(2, 0, 4, 3, 1).reshape(NCORES, L, D, T)
    hsT8 = hsT_all.reshape(NCORES, L, 2, 128, TP).astype(NPF8)
    if not np.all(np.isfinite(hsT8.astype(np.float32))):
        return None

    r = np.clip(refs.reshape(L, NCORES, BPC * Q, 3), 0.0, 1.0)
    ir = np.log(np.maximum(r, EPS) / np.maximum(1.0 - r, EPS))
    ir_all = np.zeros((NCORES, 32, TP), np.float32)
    ir_all[:, :18, :T] = ir.transpose(1, 0, 3, 2).reshape(NCORES, 18, T)
    ir_all[:, 18, :] = 1.0
    ir_all = ir_all.astype(ml_dtypes.bfloat16)

    Wmap = dict(wst=wst8, w3t=w3t, scal=sc, hb=hbm,
                b3t=np.ascontiguousarray(b3c.reshape(1, L, 10)),
                selt=sel.astype(ml_dtypes.bfloat16))
    return [dict(hsT=np.ascontiguousarray(hsT8[c]),
                 irt=np.ascontiguousarray(ir_all[c]), **Wmap)
            for c in range(NCORES)]


def _unshard(results):
    out = np.zeros((2, L, B, Q, 10), np.float32)
    for c in range(NCORES):
        vc = results[c]["o_cls"][:, :, :T]                   # [L,10,T]
        vd = results[c]["o_crd"][:, :, :T]
        vc = vc.reshape(L, 10, BPC, Q).transpose(0, 2, 3, 1)  # [L,4,Q,10]
        vd = vd.reshape(L, 10, BPC, Q).transpose(0, 2, 3, 1)[:, :, :, INV]
        out[0, :, c * BPC:(c + 1) * BPC] = vc
        out[1, :, c * BPC:(c + 1) * BPC] = vd
    return out


def kernel(**inputs):
    hs = np.asarray(inputs["hs"], np.float32)
    init_reference = np.asarray(inputs["init_reference"], np.float32)
    inter_references = np.asarray(inputs["inter_references"], np.float32)
    cls_w1 = np.asarray(inputs["cls_w1"], np.float32)
    cls_w2 = np.asarray(inputs["cls_w2"], np.float32)
    b1 = np.asarray(inputs["cls_b1"], np.float32)
    b2 = np.asarray(inputs["cls_b2"], np.float32)

    w1c = cls_w1 - cls_w1.mean(-1, keepdims=True)
    w2c = cls_w2 - cls_w2.mean(-1, keepdims=True)
    refs = np.concatenate([init_reference[None], inter_references[:L - 1]], 0)

    fast = not any(np.asarray(inputs[k], np.float32).any() for k in
                   ("cls_b1", "cls_b2", "ln1_b", "ln2_b", "reg_b1", "reg_b2"))

    in_maps = None
    if fast:
        in_maps = _prep_fast(
            hs, refs, w1c, w2c,
            np.asarray(inputs["reg_w1"], np.float32),
            np.asarray(inputs["reg_w2"], np.float32),
            inputs["ln1_g"], inputs["ln2_g"],
            inputs["cls_w3"], inputs["cls_b3"],
            inputs["reg_w3"], inputs["reg_b3"])

    if in_maps is not None:
        _cache["last_in_maps"] = in_maps
        if "ncf" not in _cache:
            _cache["ncf"] = _build_fast()
        nc = _cache["ncf"]
        res = run_bass_kernel_spmd(nc, in_maps, core_ids=list(range(NCORES)),
                                   trace=bool(os.environ.get("KTRACE")))
        _cache["last_result"] = res
        return _unshard(res.results)

    # general fallback (nonzero biases): plain numpy reference
    return _np_reference(inputs)


def _np_reference(i):
    hs = np.asarray(i["hs"], np.float32)
    h = hs.transpose(0, 2, 1, 3)
    refs = np.concatenate([np.asarray(i["init_reference"], np.float32)[None],
                           np.asarray(i["inter_references"],
                                      np.float32)[:L - 1]], 0)
    cls_o = np.zeros((L, B, Q, NC), np.float32)
    crd_o = np.zeros((L, B, Q, CS), np.float32)

    def ln(x, g, b):
        m = x.mean(-1, keepdims=True)
        v = x.var(-1, keepdims=True)
        return (x - m) / np.sqrt(v + EPS) * g + b

    sig = lambda x: 1.0 / (1.0 + np.exp(-x))
    gi = {k: np.asarray(v, np.float32) for k, v in i.items()}
    for l in range(L):
        x = np.maximum(ln(h[l] @ gi["cls_w1"][l] + gi["cls_b1"][l],
                          gi["ln1_g"][l], gi["ln1_b"][l]), 0)
        x = np.maximum(ln(x @ gi["cls_w2"][l] + gi["cls_b2"][l],
                          gi["ln2_g"][l], gi["ln2_b"][l]), 0)
        cls_o[l] = x @ gi["cls_w3"][l] + gi["cls_b3"][l]
        y = np.maximum(h[l] @ gi["reg_w1"][l] + gi["reg_b1"][l], 0)
        y = np.maximum(y @ gi["reg_w2"][l] + gi["reg_b2"][l], 0)
        tmp = y @ gi["reg_w3"][l] + gi["reg_b3"][l]
        r = np.clip(refs[l], 0.0, 1.0)
        ir = np.log(np.maximum(r, EPS) / np.maximum(1.0 - r, EPS))
        xy = sig(tmp[..., 0:2] + ir[..., 0:2])
        z = sig(tmp[..., 4:5] + ir[..., 2:3])
        cx = xy[..., 0:1] * 102.4 - 51.2
        cy = xy[..., 1:2] * 102.4 - 51.2
        cz = z * 8.0 - 5.0
        crd_o[l] = np.concatenate([cx, cy, tmp[..., 2:4], cz, tmp[..., 5:]],
                                  -1)
    return np.stack([cls_o, crd_o], 0)


# revision 9
# speedup vs baseline: 1.1228x; 1.0239x over previous
import os
import numpy as np
import ml_dtypes

import concourse.bass as bass
import concourse.tile as tile
from concourse import bacc, mybir
from concourse.bass_utils import run_bass_kernel_spmd

L, B, Q, D, NC, CS = 6, 32, 900, 256, 10, 10
EPS = 1e-5
NCORES = 8
BPC = B // NCORES          # 4 samples per core
T = BPC * Q                # 3600 tokens per core
NT = 8                     # token tiles per layer
N = 450                    # tokens per tile
TP = NT * N                # 3600 tokens, no padding
BF16 = mybir.dt.bfloat16
F32 = mybir.dt.float32
FP8 = mybir.dt.float8e4
AF = mybir.ActivationFunctionType
ALU = mybir.AluOpType
DR = mybir.MatmulPerfMode.DoubleRow
NPF8 = ml_dtypes.float8_e4m3

# head-channel permutation: rows 0,1,2 = sigmoid channels (orig 0,1,4)
PERM = [0, 1, 4, 2, 3, 5, 6, 7, 8, 9]
INV = list(np.argsort(PERM))       # coord channel c <- row INV[c]

_cache = {}


def _build_fast():
    """fp8 DoubleRow pipeline; requires all linear/LN biases zero.

    Tricks (all exact up to float rounding):
    - LN mean removal folded into centered weights (W - rowmean W).
    - ln_g folded into weight columns; LN1's rstd skipped entirely and
      LN2's rstd pulled through ReLU + head GEMM (LayerNorm is invariant
      to per-token input scaling), applied to the [10,N] head output.
    - E[z^2] computed with per-feature 1/(256 g^2) weights so the folded
      g does not disturb the variance.
    - cls_b3 added as a rank-1 (b3 x sigma) accumulate into the head psum
      before the rstd scale (sigma = 1/rstd from the same bf16 value).
    - fp8 x16 weight scaling cancels through the same invariances; the
      reg branch rescales via x16/x256 biases and w3r/256.
    """
    nc = bacc.Bacc("TRN2", target_bir_lowering=False, debug=False,
                   enable_asserts=False, num_devices=NCORES)
    hsT = nc.dram_tensor("hsT", [L, 2, 128, TP], FP8, kind="ExternalInput").ap()
    wst = nc.dram_tensor("wst", [L, 128, 4, 2, 2, 128], FP8, kind="ExternalInput").ap()
    w3t = nc.dram_tensor("w3t", [128, L, 2, 2, 10], BF16, kind="ExternalInput").ap()
    scal = nc.dram_tensor("scal", [128, L, 2, 10], BF16, kind="ExternalInput").ap()
    b3t = nc.dram_tensor("b3t", [1, L, 10], F32, kind="ExternalInput").ap()
    hb = nc.dram_tensor("hb", [10, 2 * L + 2], F32, kind="ExternalInput").ap()
    irt = nc.dram_tensor("irt", [32, TP], BF16, kind="ExternalInput").ap()
    selt = nc.dram_tensor("selt", [32, 10 * L], BF16, kind="ExternalInput").ap()
    o_cls = nc.dram_tensor("o_cls", [L, 10, TP], F32, kind="ExternalOutput").ap()
    o_crd = nc.dram_tensor("o_crd", [L, 10, TP], F32, kind="ExternalOutput").ap()

    with tile.TileContext(nc) as tc:
        with (
            tc.tile_pool(name="const", bufs=1) as cp,
            tc.tile_pool(name="stream", bufs=2) as sp,
            tc.tile_pool(name="wk", bufs=3) as wk,
            tc.tile_pool(name="ps", bufs=2, space="PSUM") as pp,
        ):
            onesc = cp.tile([1, 128], BF16)
            nc.vector.memset(onesc[:], 1.0)
            eps1 = cp.tile([10, 1], F32)
            nc.vector.memset(eps1[:], EPS)
            zer = cp.tile([128, 1], F32)
            nc.vector.memset(zer[:], 0.0)

            w_sb = []
            for l in range(L):
                wt = cp.tile([128, 4, 2, 2, 128], FP8, tag=f"w{l}", name=f"w{l}")
                (nc.sync if l < 2 else nc.scalar).dma_start(wt[:], wst[l])
                w_sb.append(wt)
            w3_sb = cp.tile([128, L, 2, 2, 10], BF16)
            nc.scalar.dma_start(w3_sb[:], w3t)
            sc_sb = cp.tile([128, L, 2, 10], BF16)
            nc.scalar.dma_start(sc_sb[:], scal)
            b3_sb = cp.tile([1, L, 10], F32)
            nc.scalar.dma_start(b3_sb[:], b3t)
            hb_sb = cp.tile([10, 2 * L + 2], F32)
            nc.scalar.dma_start(hb_sb[:], hb)
            ir_sb = cp.tile([32, TP], BF16)
            nc.scalar.dma_start(ir_sb[:], irt)
            sel_sb = cp.tile([32, 10 * L], BF16)
            nc.scalar.dma_start(sel_sb[:], selt)

            def chain(l, t, hsl, tmpa):
                    tsl = slice(t * N, (t + 1) * N)
                    # ---- reg branch ----
                    r1 = pp.tile([128, 2, 512], F32, tag="z", name="r1", bufs=3)
                    for m in range(2):
                        nc.tensor.matmul(r1[:, m, 0:N], w_sb[l][:, 2, 0:2, m, :],
                                         hsl[:, 0:2, tsl], start=True,
                                         stop=True, perf_mode=DR)
                    y1 = wk.tile([128, 2, N], FP8, tag="y1", name="y1", bufs=4)
                    nc.vector.tensor_scalar(y1[:, :, :], r1[:, :, 0:N],
                                            0.0, None, ALU.max)
                    r2 = pp.tile([128, 2, 512], F32, tag="z", name="r2", bufs=3)
                    for m in range(2):
                        nc.tensor.matmul(r2[:, m, 0:N], w_sb[l][:, 3, 0:2, m, :],
                                         y1[:, 0:2, :], start=True,
                                         stop=True, perf_mode=DR)
                    y2 = wk.tile([128, 2, N], BF16, tag="y2", name="y2", bufs=4)
                    if t % 2 == 0:
                        nc.vector.tensor_scalar(y2[:, :, :], r2[:, :, 0:N],
                                                0.0, None, ALU.max)
                    else:
                        nc.scalar.activation(y2[:, :, :], r2[:, :, 0:N],
                                             AF.Relu, bias=zer[:])
                    tps = pp.tile([128, 512], F32, tag="pb", name="tps", bufs=2)
                    for k in range(2):
                        nc.tensor.matmul(tps[0:10, 0:N], w3_sb[:, l, 1, k, :],
                                         y2[:, k, :], start=(k == 0),
                                         stop=False)
                    # adds invsig(ref) to rows 0-2 and reg_b3 to all rows
                    nc.tensor.matmul(tps[0:10, 0:N],
                                     sel_sb[:, 10 * l:10 * l + 10],
                                     ir_sb[:, tsl], start=False, stop=True,
                                     skip_group_check=True)
                    if t % 2 == 0:
                        nc.scalar.activation(tmpa[:, tsl], tps[0:10, 0:N],
                                             AF.Identity, bias=zer[0:10, :])
                    else:
                        nc.vector.tensor_copy(tmpa[:, tsl], tps[0:10, 0:N])

                    # ---- cls branch ----
                    z1 = pp.tile([128, 2, 512], F32, tag="z", name="z1", bufs=3)
                    for m in range(2):
                        nc.tensor.matmul(z1[:, m, 0:N], w_sb[l][:, 0, 0:2, m, :],
                                         hsl[:, 0:2, tsl], start=True,
                                         stop=True, perf_mode=DR)
                    x1 = wk.tile([128, 2, N], FP8, tag="x1", name="x1", bufs=4)
                    nc.scalar.activation(x1[:, :, :], z1[:, :, 0:N],
                                         AF.Relu, bias=zer[:])
                    z2 = pp.tile([128, 2, 512], F32, tag="z", name="z2", bufs=3)
                    for m in range(2):
                        nc.tensor.matmul(z2[:, m, 0:N], w_sb[l][:, 1, 0:2, m, :],
                                         x1[:, 0:2, :], start=True,
                                         stop=True, perf_mode=DR)
                    # rstd2 from g-compensated E[z^2]
                    zsq = wk.tile([128, 2, N], BF16, tag="zsq", name="zsq", bufs=4)
                    nc.scalar.activation(zsq[:, :, :], z2[:, :, 0:N],
                                         AF.Square, bias=zer[:])
                    var = pp.tile([128, 512], F32, tag="pb", name="var", bufs=2)
                    for m in range(2):
                        nc.tensor.matmul(var[0:10, 0:N], sc_sb[:, l, m, :],
                                         zsq[:, m, :], start=(m == 0),
                                         stop=(m == 1))
                    srt = wk.tile([10, N], F32, tag="srt", name="srt", bufs=3)
                    nc.scalar.activation(srt[:], var[0:10, 0:N], AF.Sqrt,
                                         bias=eps1[:])
                    rstd = wk.tile([10, N], F32, tag="rstd", name="rstd",
                                   bufs=3)
                    nc.vector.reciprocal_approx_fast(rstd[:], srt[:])
                    x2 = wk.tile([128, 2, N], BF16, tag="x2", name="x2", bufs=4)
                    nc.vector.tensor_scalar(x2[:, :, :], z2[:, :, 0:N],
                                            0.0, None, ALU.max)
                    cps = pp.tile([128, 512], F32, tag="pb", name="cps", bufs=2)
                    for k in range(2):
                        nc.tensor.matmul(cps[0:10, 0:N], w3_sb[:, l, 0, k, :],
                                         x2[:, k, :], start=(k == 0),
                                         stop=False)
                    # + cls_b3 (x) sigma, cancelled by the rstd scale below
                    nc.tensor.matmul(cps[0:10, 0:N], b3_sb[0:1, l, :],
                                     srt[0:1, :], start=False, stop=True,
                                     skip_group_check=True)
                    cls_sb = wk.tile([10, N], F32, tag="cls", name="cls")
                    nc.vector.tensor_tensor(cls_sb[:], cps[0:10, 0:N],
                                            rstd[:], ALU.mult)
                    if t % 2 == 0:
                        nc.scalar.dma_start(o_cls[l, :, tsl], cls_sb[:])
                    else:
                        nc.sync.dma_start(o_cls[l, :, tsl], cls_sb[:])

            def finish_layer(l, tmpa):
                for h in range(2):
                    hsl2 = slice(h * (TP // 2), (h + 1) * (TP // 2))
                    sig = tmpa[0:3, hsl2]
                    nc.scalar.activation(sig, sig, AF.Exp, scale=-1.0,
                                         bias=zer[0:3, :])
                    nc.gpsimd.tensor_scalar(sig, sig, 1.0, None, ALU.add)
                    nc.vector.reciprocal_approx_fast(sig, sig)
                    nc.gpsimd.tensor_scalar(sig, sig,
                                            hb_sb[0:3, 2 * L:2 * L + 1],
                                            hb_sb[0:3, 2 * L + 1:2 * L + 2],
                                            ALU.mult, ALU.add)
                    nc.sync.dma_start(o_crd[l, :, hsl2], tmpa[:, hsl2])

            for lp in range(0, L, 2):
                hs_t, tm_t = [], []
                for l in (lp, lp + 1):
                    hsl = sp.tile([128, 2, TP], FP8, tag="hs", name=f"hs{l}",
                                  bufs=4)
                    if l == 0:
                        for k in range(2):
                            for q in range(4):
                                qs = slice(q * (TP // 4), (q + 1) * (TP // 4))
                                nc.sync.dma_start(hsl[:, k, qs], hsT[l, k, :, qs])
                    else:
                        for k in range(2):
                            nc.sync.dma_start(hsl[:, k, :], hsT[l, k])
                    tmpa = sp.tile([10, TP], F32, tag="tmpa",
                                   name=f"tmpa{l}", bufs=4)
                    hs_t.append(hsl)
                    tm_t.append(tmpa)
                for t in range(NT):
                    chain(lp, t, hs_t[0], tm_t[0])
                    chain(lp + 1, NT - 1 - t, hs_t[1], tm_t[1])
                finish_layer(lp, tm_t[0])
                finish_layer(lp + 1, tm_t[1])

    nc.compile()
    return nc


def _prep_fast(hs, refs, w1c, w2c, reg_w1, reg_w2, ln1_g, ln2_g,
               cls_w3, cls_b3, reg_w3, reg_b3):
    g1 = np.asarray(ln1_g, np.float32).reshape(L, 1, D)
    g2 = np.asarray(ln2_g, np.float32).reshape(L, 1, D)
    ws = np.stack([w1c * g1, w2c * g2, reg_w1, reg_w2], 1)   # [L,4,256,256]
    wst = ws.reshape(L, 4, 2, 128, 2, 128).transpose(0, 3, 1, 2, 4, 5)
    wst8 = (wst * 16.0).astype(NPF8)
    if not np.all(np.isfinite(wst8.astype(np.float32))):
        return None
    wst8 = np.ascontiguousarray(wst8)

    w3c = np.asarray(cls_w3, np.float32)                     # [L,256,10]
    w3r = np.asarray(reg_w3, np.float32)[:, :, PERM] / 256.0
    w3s = np.stack([w3c, w3r], 1).reshape(L, 2, 2, 128, 10)
    w3t = np.ascontiguousarray(
        w3s.transpose(3, 0, 1, 2, 4).astype(ml_dtypes.bfloat16))

    # scal cols = 1/(256 g2^2) per k-chunk, bf16 (matmul lhsT operand)
    sc = np.zeros((128, L, 2, 10), np.float32)
    g2sq = 1.0 / (256.0 * np.maximum(np.abs(g2.reshape(L, D)), 1e-30) ** 2)
    sc[:, :, 0, :] = g2sq[:, 0:128].T[:, :, None]
    sc[:, :, 1, :] = g2sq[:, 128:256].T[:, :, None]
    sc = sc.astype(ml_dtypes.bfloat16)

    b3c = np.asarray(cls_b3, np.float32).reshape(L, 10)
    b3r = np.asarray(reg_b3, np.float32).reshape(L, 10)[:, PERM]
    hbm = np.zeros((10, 2 * L + 2), np.float32)
    hbm[:, 2 * L] = [102.4, 102.4, 8.0] + [1.0] * 7
    hbm[:, 2 * L + 1] = [-51.2, -51.2, -5.0] + [0.0] * 7

    sel = np.zeros((32, 10 * L), np.float32)
    for l in range(L):
        for c in range(3):
            sel[3 * l + c, 10 * l + c] = 1.0
        sel[18, 10 * l:10 * l + 10] = b3r[l]

    h = hs.reshape(L, Q, NCORES, BPC, D)
    hsT_all = np.zeros((NCORES, L, D, TP), np.float32)
    hsT_all[:, :, :, :T] = h.transpose(2, 0, 4, 3, 1).reshape(NCORES, L, D, T)
    hsT8 = hsT_all.reshape(NCORES, L, 2, 128, TP).astype(NPF8)
    if not np.all(np.isfinite(hsT8.astype(np.float32))):
        return None

    r = np.clip(refs.reshape(L, NCORES, BPC * Q, 3), 0.0, 1.0)
    ir = np.log(np.maximum(r, EPS) / np.maximum(1.0 - r, EPS))
    ir_all = np.zeros((NCORES, 32, TP), np.float32)
    ir_all[:, :18, :T] = ir.transpose(1, 0, 3, 2).reshape(NCORES, 18, T)
    ir_all[:, 18, :] = 1.0
    ir_all = ir_all.astype(ml_dtypes.bfloat16)

    Wmap = dict(wst=wst8, w3t=w3t, scal=sc, hb=hbm,
                b3t=np.ascontiguousarray(b3c.reshape(1, L, 10)),
                selt=sel.astype(ml_dtypes.bfloat16))
    return [dict(hsT=np.ascontiguousarray(hsT8[c]),
                 irt=np.ascontiguousarray(ir_all[c]), **Wmap)
            for c in range(NCORES)]


def _unshard(results):
    out = np.zeros((2, L, B, Q, 10), np.float32)
    for c in range(NCORES):
        vc = results[c]["o_cls"][:, :, :T]                   # [L,10,T]
        vd = results[c]["o_crd"][:, :, :T]
        vc = vc.reshape(L, 10, BPC, Q).transpose(0, 2, 3, 1)  # [L,4,Q,10]
        vd = vd.reshape(L, 10, BPC, Q).transpose(0, 2, 3, 1)[:, :, :, INV]
        out[0, :, c * BPC:(c + 1) * BPC] = vc
        out[1, :, c * BPC:(c + 1) * BPC] = vd
    return out


def kernel(**inputs):
    hs = np.asarray(inputs["hs"], np.float32)
    init_reference = np.asarray(inputs["init_reference"], np.float32)
    inter_references = np.asarray(inputs["inter_references"], np.float32)
    cls_w1 = np.asarray(inputs["cls_w1"], np.float32)
    cls_w2 = np.asarray(inputs["cls_w2"], np.float32)
    b1 = np.asarray(inputs["cls_b1"], np.float32)
    b2 = np.asarray(inputs["cls_b2"], np.float32)

    w1c = cls_w1 - cls_w1.mean(-1, keepdims=True)
    w2c = cls_w2 - cls_w2.mean(-1, keepdims=True)
    refs = np.concatenate([init_reference[None], inter_references[:L - 1]], 0)

    fast = not any(np.asarray(inputs[k], np.float32).any() for k in
                   ("cls_b1", "cls_b2", "ln1_b", "ln2_b", "reg_b1", "reg_b2"))

    in_maps = None
    if fast:
        in_maps = _prep_fast(
            hs, refs, w1c, w2c,
            np.asarray(inputs["reg_w1"], np.float32),
            np.asarray(inputs["reg_w2"], np.float32),
            inputs["ln1_g"], inputs["ln2_g"],
            inputs["cls_w3"], inputs["cls_b3"],
            inputs["reg_w3"], inputs["reg_b3"])

    if in_maps is not None:
        _cache["last_in_maps"] = in_maps
        if "ncf" not in _cache:
            _cache["ncf"] = _build_fast()
        nc = _cache["ncf"]
        res = run_bass_kernel_spmd(nc, in_maps, core_ids=list(range(NCORES)),
                                   trace=bool(os.environ.get("KTRACE")))
        _cache["last_result"] = res
        return _unshard(res.results)

    # general fallback (nonzero biases): plain numpy reference
    return _np_reference(inputs)


def _np_reference(i):
    hs = np.asarray(i["hs"], np.float32)
    h = hs.transpose(0, 2, 1, 3)
    refs = np.concatenate([np.asarray(i["init_reference"], np.float32)[None],
                           np.asarray(i["inter_references"],
                                      np.float32)[:L - 1]], 0)
    cls_o = np.zeros((L, B, Q, NC), np.float32)
    crd_o = np.zeros((L, B, Q, CS), np.float32)

    def ln(x, g, b):
        m = x.mean(-1, keepdims=True)
        v = x.var(-1, keepdims=True)
        return (x - m) / np.sqrt(v + EPS) * g + b

    sig = lambda x: 1.0 / (1.0 + np.exp(-x))
    gi = {k: np.asarray(v, np.float32) for k, v in i.items()}
    for l in range(L):
        x = np.maximum(ln(h[l] @ gi["cls_w1"][l] + gi["cls_b1"][l],
                          gi["ln1_g"][l], gi["ln1_b"][l]), 0)
        x = np.maximum(ln(x @ gi["cls_w2"][l] + gi["cls_b2"][l],
                          gi["ln2_g"][l], gi["ln2_b"][l]), 0)
        cls_o[l] = x @ gi["cls_w3"][l] + gi["cls_b3"][l]
        y = np.maximum(h[l] @ gi["reg_w1"][l] + gi["reg_b1"][l], 0)
        y = np.maximum(y @ gi["reg_w2"][l] + gi["reg_b2"][l], 0)
        tmp = y @ gi["reg_w3"][l] + gi["reg_b3"][l]
        r = np.clip(refs[l], 0.0, 1.0)
        ir = np.log(np.maximum(r, EPS) / np.maximum(1.0 - r, EPS))
        xy = sig(tmp[..., 0:2] + ir[..., 0:2])
        z = sig(tmp[..., 4:5] + ir[..., 2:3])
        cx = xy[..., 0:1] * 102.4 - 51.2
        cy = xy[..., 1:2] * 102.4 - 51.2
        cz = z * 8.0 - 5.0
        crd_o[l] = np.concatenate([cx, cy, tmp[..., 2:4], cz, tmp[..., 5:]],
                                  -1)
    return np.stack([cls_o, crd_o], 0)


# revision 10
# speedup vs baseline: 1.1472x; 1.0218x over previous
import os
import numpy as np
import ml_dtypes

import concourse.bass as bass
import concourse.tile as tile
from concourse import bacc, mybir
from concourse.bass_utils import run_bass_kernel_spmd

L, B, Q, D, NC, CS = 6, 32, 900, 256, 10, 10
EPS = 1e-5
NCORES = 8
BPC = B // NCORES          # 4 samples per core
T = BPC * Q                # 3600 tokens per core
NT = 8                     # token tiles per layer
N = 450                    # tokens per tile
TP = NT * N                # 3600 tokens, no padding
BF16 = mybir.dt.bfloat16
F32 = mybir.dt.float32
FP8 = mybir.dt.float8e4
AF = mybir.ActivationFunctionType
ALU = mybir.AluOpType
DR = mybir.MatmulPerfMode.DoubleRow
NPF8 = ml_dtypes.float8_e4m3

# head-channel permutation: rows 0,1,2 = sigmoid channels (orig 0,1,4)
PERM = [0, 1, 4, 2, 3, 5, 6, 7, 8, 9]
INV = list(np.argsort(PERM))       # coord channel c <- row INV[c]

_cache = {}


def _build_fast():
    """fp8 DoubleRow pipeline; requires all linear/LN biases zero.

    Tricks (all exact up to float rounding):
    - LN mean removal folded into centered weights (W - rowmean W).
    - ln_g folded into weight columns; LN1's rstd skipped entirely and
      LN2's rstd pulled through ReLU + head GEMM (LayerNorm is invariant
      to per-token input scaling), applied to the [10,N] head output.
    - E[z^2] computed with per-feature 1/(256 g^2) weights so the folded
      g does not disturb the variance.
    - cls_b3 added as a rank-1 (b3 x sigma) accumulate into the head psum
      before the rstd scale (sigma = 1/rstd from the same bf16 value).
    - fp8 x16 weight scaling cancels through the same invariances; the
      reg branch rescales via x16/x256 biases and w3r/256.
    """
    nc = bacc.Bacc("TRN2", target_bir_lowering=False, debug=False,
                   enable_asserts=False, num_devices=NCORES)
    hsT = nc.dram_tensor("hsT", [L, 2, 128, TP], FP8, kind="ExternalInput").ap()
    wst = nc.dram_tensor("wst", [L, 128, 4, 2, 2, 128], FP8, kind="ExternalInput").ap()
    w3t = nc.dram_tensor("w3t", [128, L, 2, 2, 10], BF16, kind="ExternalInput").ap()
    scal = nc.dram_tensor("scal", [128, L, 2, 10], BF16, kind="ExternalInput").ap()
    b3t = nc.dram_tensor("b3t", [1, L, 10], F32, kind="ExternalInput").ap()
    hb = nc.dram_tensor("hb", [10, 2 * L + 2], F32, kind="ExternalInput").ap()
    irt = nc.dram_tensor("irt", [32, TP], BF16, kind="ExternalInput").ap()
    selt = nc.dram_tensor("selt", [32, 10 * L], BF16, kind="ExternalInput").ap()
    o_cls = nc.dram_tensor("o_cls", [L, 10, TP], F32, kind="ExternalOutput").ap()
    o_crd = nc.dram_tensor("o_crd", [L, 10, TP], F32, kind="ExternalOutput").ap()

    with tile.TileContext(nc) as tc:
        with (
            tc.tile_pool(name="const", bufs=1) as cp,
            tc.tile_pool(name="stream", bufs=2) as sp,
            tc.tile_pool(name="wk", bufs=3) as wk,
            tc.tile_pool(name="ps", bufs=2, space="PSUM") as pp,
        ):
            onesc = cp.tile([1, 128], BF16)
            nc.vector.memset(onesc[:], 1.0)
            eps1 = cp.tile([10, 1], F32)
            nc.vector.memset(eps1[:], EPS)
            zer = cp.tile([128, 1], F32)
            nc.vector.memset(zer[:], 0.0)

            w_sb = []
            for l in range(L):
                wt = cp.tile([128, 4, 2, 2, 128], FP8, tag=f"w{l}", name=f"w{l}")
                (nc.sync if l < 2 else nc.scalar).dma_start(wt[:], wst[l])
                w_sb.append(wt)
            w3_sb = cp.tile([128, L, 2, 2, 10], BF16)
            nc.scalar.dma_start(w3_sb[:], w3t)
            sc_sb = cp.tile([128, L, 2, 10], BF16)
            nc.scalar.dma_start(sc_sb[:], scal)
            b3_sb = cp.tile([1, L, 10], F32)
            nc.scalar.dma_start(b3_sb[:], b3t)
            hb_sb = cp.tile([10, 2 * L + 2], F32)
            nc.scalar.dma_start(hb_sb[:], hb)
            ir_sb = cp.tile([32, TP], BF16)
            nc.scalar.dma_start(ir_sb[:], irt)
            sel_sb = cp.tile([32, 10 * L], BF16)
            nc.scalar.dma_start(sel_sb[:], selt)

            def chain(l, t, hsl, tmpa):
                    tsl = slice(t * N, (t + 1) * N)
                    # ---- reg branch ----
                    r1 = pp.tile([128, 2, 512], F32, tag="z", name="r1", bufs=3)
                    for m in range(2):
                        nc.tensor.matmul(r1[:, m, 0:N], w_sb[l][:, 2, 0:2, m, :],
                                         hsl[:, 0:2, tsl], start=True,
                                         stop=True, perf_mode=DR)
                    y1 = wk.tile([128, 2, N], FP8, tag="y1", name="y1", bufs=4)
                    nc.vector.tensor_scalar(y1[:, :, :], r1[:, :, 0:N],
                                            0.0, None, ALU.max)
                    r2 = pp.tile([128, 2, 512], F32, tag="z", name="r2", bufs=3)
                    for m in range(2):
                        nc.tensor.matmul(r2[:, m, 0:N], w_sb[l][:, 3, 0:2, m, :],
                                         y1[:, 0:2, :], start=True,
                                         stop=True, perf_mode=DR)
                    y2 = wk.tile([128, 2, N], BF16, tag="y2", name="y2", bufs=4)
                    if t % 2 == 0:
                        nc.vector.tensor_scalar(y2[:, :, :], r2[:, :, 0:N],
                                                0.0, None, ALU.max)
                    else:
                        nc.scalar.activation(y2[:, :, :], r2[:, :, 0:N],
                                             AF.Relu, bias=zer[:])
                    tps = pp.tile([128, 512], F32, tag="pb", name="tps", bufs=2)
                    for k in range(2):
                        nc.tensor.matmul(tps[0:10, 0:N], w3_sb[:, l, 1, k, :],
                                         y2[:, k, :], start=(k == 0),
                                         stop=False)
                    # adds invsig(ref) to rows 0-2 and reg_b3 to all rows
                    nc.tensor.matmul(tps[0:10, 0:N],
                                     sel_sb[:, 10 * l:10 * l + 10],
                                     ir_sb[:, tsl], start=False, stop=True,
                                     skip_group_check=True)
                    if t % 2 == 0:
                        nc.scalar.activation(tmpa[:, tsl], tps[0:10, 0:N],
                                             AF.Identity, bias=zer[0:10, :])
                    else:
                        nc.vector.tensor_copy(tmpa[:, tsl], tps[0:10, 0:N])

                    # ---- cls branch ----
                    z1 = pp.tile([128, 2, 512], F32, tag="z", name="z1", bufs=3)
                    for m in range(2):
                        nc.tensor.matmul(z1[:, m, 0:N], w_sb[l][:, 0, 0:2, m, :],
                                         hsl[:, 0:2, tsl], start=True,
                                         stop=True, perf_mode=DR)
                    x1 = wk.tile([128, 2, N], FP8, tag="x1", name="x1", bufs=4)
                    nc.scalar.activation(x1[:, :, :], z1[:, :, 0:N],
                                         AF.Relu, bias=zer[:])
                    z2 = pp.tile([128, 2, 512], F32, tag="z", name="z2", bufs=3)
                    for m in range(2):
                        nc.tensor.matmul(z2[:, m, 0:N], w_sb[l][:, 1, 0:2, m, :],
                                         x1[:, 0:2, :], start=True,
                                         stop=True, perf_mode=DR)
                    # rstd2 from g-compensated E[z^2]
                    zsq = wk.tile([128, 2, N], BF16, tag="zsq", name="zsq", bufs=4)
                    nc.scalar.activation(zsq[:, :, :], z2[:, :, 0:N],
                                         AF.Square, bias=zer[:])
                    var = pp.tile([128, 512], F32, tag="pb", name="var", bufs=2)
                    for m in range(2):
                        nc.tensor.matmul(var[0:10, 0:N], sc_sb[:, l, m, :],
                                         zsq[:, m, :], start=(m == 0),
                                         stop=(m == 1))
                    srt = wk.tile([10, N], F32, tag="srt", name="srt", bufs=3)
                    nc.scalar.activation(srt[:], var[0:10, 0:N], AF.Sqrt,
                                         bias=eps1[:])
                    rstd = wk.tile([10, N], F32, tag="rstd", name="rstd",
                                   bufs=3)
                    nc.vector.reciprocal_approx_fast(rstd[:], srt[:])
                    x2 = wk.tile([128, 2, N], BF16, tag="x2", name="x2", bufs=4)
                    nc.vector.tensor_scalar(x2[:, :, :], z2[:, :, 0:N],
                                            0.0, None, ALU.max)
                    cps = pp.tile([128, 512], F32, tag="pb", name="cps", bufs=2)
                    for k in range(2):
                        nc.tensor.matmul(cps[0:10, 0:N], w3_sb[:, l, 0, k, :],
                                         x2[:, k, :], start=(k == 0),
                                         stop=False)
                    # + cls_b3 (x) sigma, cancelled by the rstd scale below
                    nc.tensor.matmul(cps[0:10, 0:N], b3_sb[0:1, l, :],
                                     srt[0:1, :], start=False, stop=True,
                                     skip_group_check=True)
                    cls_sb = wk.tile([10, N], F32, tag="cls", name="cls")
                    nc.vector.tensor_tensor(cls_sb[:], cps[0:10, 0:N],
                                            rstd[:], ALU.mult)
                    nc.sync.dma_start(o_cls[l, :, tsl], cls_sb[:])

            def finish_layer(l, tmpa):
                for h in range(2):
                    hsl2 = slice(h * (TP // 2), (h + 1) * (TP // 2))
                    sig = tmpa[0:3, hsl2]
                    nc.scalar.activation(sig, sig, AF.Exp, scale=-1.0,
                                         bias=zer[0:3, :])
                    nc.gpsimd.tensor_scalar(sig, sig, 1.0, None, ALU.add)
                    nc.vector.reciprocal_approx_fast(sig, sig)
                    nc.gpsimd.tensor_scalar(sig, sig,
                                            hb_sb[0:3, 2 * L:2 * L + 1],
                                            hb_sb[0:3, 2 * L + 1:2 * L + 2],
                                            ALU.mult, ALU.add)
                    nc.sync.dma_start(o_crd[l, :, hsl2], tmpa[:, hsl2])

            for lp in range(0, L, 2):
                hs_t, tm_t = [], []
                for l in (lp, lp + 1):
                    hsl = sp.tile([128, 2, TP], FP8, tag="hs", name=f"hs{l}",
                                  bufs=4)
                    if l == 0:
                        for k in range(2):
                            for q in range(4):
                                qs = slice(q * (TP // 4), (q + 1) * (TP // 4))
                                nc.sync.dma_start(hsl[:, k, qs], hsT[l, k, :, qs])
                    else:
                        for k in range(2):
                            nc.sync.dma_start(hsl[:, k, :], hsT[l, k])
                    tmpa = sp.tile([10, TP], F32, tag="tmpa",
                                   name=f"tmpa{l}", bufs=4)
                    hs_t.append(hsl)
                    tm_t.append(tmpa)
                for t in range(NT):
                    chain(lp, t, hs_t[0], tm_t[0])
                    chain(lp + 1, NT - 1 - t, hs_t[1], tm_t[1])
                finish_layer(lp, tm_t[0])
                finish_layer(lp + 1, tm_t[1])

    nc.compile()
    return nc


def _prep_fast(hs, refs, w1c, w2c, reg_w1, reg_w2, ln1_g, ln2_g,
               cls_w3, cls_b3, reg_w3, reg_b3):
    g1 = np.asarray(ln1_g, np.float32).reshape(L, 1, D)
    g2 = np.asarray(ln2_g, np.float32).reshape(L, 1, D)
    ws = np.stack([w1c * g1, w2c * g2, reg_w1, reg_w2], 1)   # [L,4,256,256]
    wst = ws.reshape(L, 4, 2, 128, 2, 128).transpose(0, 3, 1, 2, 4, 5)
    wst8 = (wst * 16.0).astype(NPF8)
    if not np.all(np.isfinite(wst8.astype(np.float32))):
        return None
    wst8 = np.ascontiguousarray(wst8)

    w3c = np.asarray(cls_w3, np.float32)                     # [L,256,10]
    w3r = np.asarray(reg_w3, np.float32)[:, :, PERM] / 256.0
    w3s = np.stack([w3c, w3r], 1).reshape(L, 2, 2, 128, 10)
    w3t = np.ascontiguousarray(
        w3s.transpose(3, 0, 1, 2, 4).astype(ml_dtypes.bfloat16))

    # scal cols = 1/(256 g2^2) per k-chunk, bf16 (matmul lhsT operand)
    sc = np.zeros((128, L, 2, 10), np.float32)
    g2sq = 1.0 / (256.0 * np.maximum(np.abs(g2.reshape(L, D)), 1e-30) ** 2)
    sc[:, :, 0, :] = g2sq[:, 0:128].T[:, :, None]
    sc[:, :, 1, :] = g2sq[:, 128:256].T[:, :, None]
    sc = sc.astype(ml_dtypes.bfloat16)

    b3c = np.asarray(cls_b3, np.float32).reshape(L, 10)
    b3r = np.asarray(reg_b3, np.float32).reshape(L, 10)[:, PERM]
    hbm = np.zeros((10, 2 * L + 2), np.float32)
    hbm[:, 2 * L] = [102.4, 102.4, 8.0] + [1.0] * 7
    hbm[:, 2 * L + 1] = [-51.2, -51.2, -5.0] + [0.0] * 7

    sel = np.zeros((32, 10 * L), np.float32)
    for l in range(L):
        for c in range(3):
            sel[3 * l + c, 10 * l + c] = 1.0
        sel[18, 10 * l:10 * l + 10] = b3r[l]

    h = hs.reshape(L, Q, NCORES, BPC, D)
    hsT_all = np.zeros((NCORES, L, D, TP), np.float32)
    hsT_all[:, :, :, :T] = h.transpose(2, 0, 4, 3, 1).reshape(NCORES, L, D, T)
    hsT8 = hsT_all.reshape(NCORES, L, 2, 128, TP).astype(NPF8)
    if not np.all(np.isfinite(hsT8.astype(np.float32))):
        return None

    r = np.clip(refs.reshape(L, NCORES, BPC * Q, 3), 0.0, 1.0)
    ir = np.log(np.maximum(r, EPS) / np.maximum(1.0 - r, EPS))
    ir_all = np.zeros((NCORES, 32, TP), np.float32)
    ir_all[:, :18, :T] = ir.transpose(1, 0, 3, 2).reshape(NCORES, 18, T)
    ir_all[:, 18, :] = 1.0
    ir_all = ir_all.astype(ml_dtypes.bfloat16)

    Wmap = dict(wst=wst8, w3t=w3t, scal=sc, hb=hbm,
                b3t=np.ascontiguousarray(b3c.reshape(1, L, 10)),
                selt=sel.astype(ml_dtypes.bfloat16))
    return [dict(hsT=np.ascontiguousarray(hsT8[c]),
                 irt=np.ascontiguousarray(ir_all[c]), **Wmap)
            for c in range(NCORES)]


def _unshard(results):
    out = np.zeros((2, L, B, Q, 10), np.float32)
    for c in range(NCORES):
        vc = results[c]["o_cls"][:, :, :T]                   # [L,10,T]
        vd = results[c]["o_crd"][:, :, :T]
        vc = vc.reshape(L, 10, BPC, Q).transpose(0, 2, 3, 1)  # [L,4,Q,10]
        vd = vd.reshape(L, 10, BPC, Q).transpose(0, 2, 3, 1)[:, :, :, INV]
        out[0, :, c * BPC:(c + 1) * BPC] = vc
        out[1, :, c * BPC:(c + 1) * BPC] = vd
    return out


def kernel(**inputs):
    hs = np.asarray(inputs["hs"], np.float32)
    init_reference = np.asarray(inputs["init_reference"], np.float32)
    inter_references = np.asarray(inputs["inter_references"], np.float32)
    cls_w1 = np.asarray(inputs["cls_w1"], np.float32)
    cls_w2 = np.asarray(inputs["cls_w2"], np.float32)
    b1 = np.asarray(inputs["cls_b1"], np.float32)
    b2 = np.asarray(inputs["cls_b2"], np.float32)

    w1c = cls_w1 - cls_w1.mean(-1, keepdims=True)
    w2c = cls_w2 - cls_w2.mean(-1, keepdims=True)
    refs = np.concatenate([init_reference[None], inter_references[:L - 1]], 0)

    fast = not any(np.asarray(inputs[k], np.float32).any() for k in
                   ("cls_b1", "cls_b2", "ln1_b", "ln2_b", "reg_b1", "reg_b2"))

    in_maps = None
    if fast:
        in_maps = _prep_fast(
            hs, refs, w1c, w2c,
            np.asarray(inputs["reg_w1"], np.float32),
            np.asarray(inputs["reg_w2"], np.float32),
            inputs["ln1_g"], inputs["ln2_g"],
            inputs["cls_w3"], inputs["cls_b3"],
            inputs["reg_w3"], inputs["reg_b3"])

    if in_maps is not None:
        _cache["last_in_maps"] = in_maps
        if "ncf" not in _cache:
            _cache["ncf"] = _build_fast()
        nc = _cache["ncf"]
        res = run_bass_kernel_spmd(nc, in_maps, core_ids=list(range(NCORES)),
                                   trace=bool(os.environ.get("KTRACE")))
        _cache["last_result"] = res
        return _unshard(res.results)

    # general fallback (nonzero biases): plain numpy reference
    return _np_reference(inputs)


def _np_reference(i):
    hs = np.asarray(i["hs"], np.float32)
    h = hs.transpose(0, 2, 1, 3)
    refs = np.concatenate([np.asarray(i["init_reference"], np.float32)[None],
                           np.asarray(i["inter_references"],
                                      np.float32)[:L - 1]], 0)
    cls_o = np.zeros((L, B, Q, NC), np.float32)
    crd_o = np.zeros((L, B, Q, CS), np.float32)

    def ln(x, g, b):
        m = x.mean(-1, keepdims=True)
        v = x.var(-1, keepdims=True)
        return (x - m) / np.sqrt(v + EPS) * g + b

    sig = lambda x: 1.0 / (1.0 + np.exp(-x))
    gi = {k: np.asarray(v, np.float32) for k, v in i.items()}
    for l in range(L):
        x = np.maximum(ln(h[l] @ gi["cls_w1"][l] + gi["cls_b1"][l],
                          gi["ln1_g"][l], gi["ln1_b"][l]), 0)
        x = np.maximum(ln(x @ gi["cls_w2"][l] + gi["cls_b2"][l],
                          gi["ln2_g"][l], gi["ln2_b"][l]), 0)
        cls_o[l] = x @ gi["cls_w3"][l] + gi["cls_b3"][l]
        y = np.maximum(h[l] @ gi["reg_w1"][l] + gi["reg_b1"][l], 0)
        y = np.maximum(y @ gi["reg_w2"][l] + gi["reg_b2"][l], 0)
        tmp = y @ gi["reg_w3"][l] + gi["reg_b3"][l]
        r = np.clip(refs[l], 0.0, 1.0)
        ir = np.log(np.maximum(r, EPS) / np.maximum(1.0 - r, EPS))
        xy = sig(tmp[..., 0:2] + ir[..., 0:2])
        z = sig(tmp[..., 4:5] + ir[..., 2:3])
        cx = xy[..., 0:1] * 102.4 - 51.2
        cy = xy[..., 1:2] * 102.4 - 51.2
        cz = z * 8.0 - 5.0
        crd_o[l] = np.concatenate([cx, cy, tmp[..., 2:4], cz, tmp[..., 5:]],
                                  -1)
    return np.stack([cls_o, crd_o], 0)
